# revision 16
# baseline (speedup 1.0000x reference)
"""DPP attention-3 Trainium2 kernel.

Data-parallel across 8 NeuronCores: one batch element per core; all
weights replicated.  The reference's [B,L,L,L] det_values tensor is never
materialized: since K = s2 @ s2.T is exactly symmetric, the k-reduction of
the 3x3 determinants collapses to

    marginal[i,j] = S0*(d_i d_j - K_ij^2) - d_i a_j - a_i d_j + 2 K_ij A_ij

with  A = K diag(w) K,  a = diag(A),  d = diag(K),  S0 = sum_k w_k d_k.

Everything is computed in the transposed [j, i] orientation (score is
symmetric up to the additive mask, which is fed pre-transposed from the
host), so the exp'd scores are directly usable as the stationary operand
of the context matmul and the row-sums Z fall out of an extra all-ones
column appended to the value matrix.
"""

import numpy as np

B, L, H = 8, 160, 64
N_CORES = 8
EPS = 1e-12
CHUNKS = [(0, 128), (128, 32)]  # partition chunks covering L=160

_programs = {}  # (flags, scale) -> nc


def _build_program(use_mask, use_w, use_bde, use_ln, scale):
    import concourse.bass as bass
    import concourse.tile as tile
    from concourse import bacc, mybir
    from concourse.masks import make_identity

    f32 = mybir.dt.float32
    Alu = mybir.AluOpType
    Act = mybir.ActivationFunctionType

    nc = bacc.Bacc(
        "TRN2",
        target_bir_lowering=False,
        debug=False,
        enable_asserts=False,
        num_devices=N_CORES,
    )

    inv_h4 = float(H ** -0.25)

    x_d = nc.dram_tensor("x", [L, H], f32, kind="ExternalInput").ap()
    wqt_d = nc.dram_tensor("wqt", [H, H], f32, kind="ExternalInput").ap()
    wvt_d = nc.dram_tensor("wvt", [H, H], f32, kind="ExternalInput").ap()
    wdt_d = nc.dram_tensor("wdt", [H, H], f32, kind="ExternalInput").ap()
    bqp_d = nc.dram_tensor("bqp", [H, 1], f32, kind="ExternalInput").ap()
    maskt_d = wrow_d = bde_d = lnw_d = lnb_d = None
    if use_mask:
        maskt_d = nc.dram_tensor("maskt", [L, L], f32, kind="ExternalInput").ap()
    if use_w:
        wrow_d = nc.dram_tensor("wrow", [1, L], f32, kind="ExternalInput").ap()
    if use_bde:
        bde_d = nc.dram_tensor("bde", [1, H], f32, kind="ExternalInput").ap()
    if use_ln:
        lnw_d = nc.dram_tensor("lnw", [1, H], f32, kind="ExternalInput").ap()
        lnb_d = nc.dram_tensor("lnb", [1, H], f32, kind="ExternalInput").ap()
    y_d = nc.dram_tensor("y", [L, H], f32, kind="ExternalOutput").ap()

    # scale_factor is an input tensor; bake the score arithmetic around a
    # runtime [1,1] scalar so any value works.
    with tile.TileContext(nc) as tc:
        from contextlib import ExitStack

        with ExitStack() as ctx:
            con = ctx.enter_context(tc.tile_pool(name="con", bufs=1))
            wk = ctx.enter_context(tc.tile_pool(name="wk", bufs=1))
            ppb = ctx.enter_context(tc.tile_pool(name="ppb", bufs=4, space="PSUM"))
            pps = ctx.enter_context(tc.tile_pool(name="pps", bufs=2, space="PSUM"))
            ppm = ctx.enter_context(tc.tile_pool(name="ppm", bufs=2, space="PSUM"))

            # --- constants / inputs ---
            ident = con.tile([128, 128], f32)
            make_identity(nc, ident[:])
            ones64 = con.tile([H, 1], f32)
            nc.vector.memset(ones64[:], 1.0)
            ones128 = con.tile([128, 1], f32)
            nc.vector.memset(ones128[:], 1.0)
            epsc = con.tile([128, 1], f32)
            nc.vector.memset(epsc[:], EPS)

            wqt = con.tile([H, H], f32)
            nc.sync.dma_start(out=wqt[:], in_=wqt_d)
            wvt = con.tile([H, H], f32)
            nc.sync.dma_start(out=wvt[:], in_=wvt_d)
            wdt = con.tile([H, H], f32)
            nc.sync.dma_start(out=wdt[:], in_=wdt_d)
            bqp = con.tile([H, 1], f32)
            nc.sync.dma_start(out=bqp[:], in_=bqp_d)

            xc = []
            for off, p in CHUNKS:
                t = con.tile([p, H], f32, tag=f"x{off}")
                nc.sync.dma_start(out=t[:], in_=x_d[off : off + p, :])
                xc.append(t)
            masktc = []
            if use_mask:
                for off, p in CHUNKS:
                    t = con.tile([p, L], f32, tag=f"mt{off}")
                    nc.sync.dma_start(out=t[:], in_=maskt_d[off : off + p, :])
                    masktc.append(t)
            if use_w:
                wrow = con.tile([1, L], f32)
                nc.sync.dma_start(out=wrow[:], in_=wrow_d)
            if use_bde:
                bde_r = con.tile([1, H], f32)
                nc.sync.dma_start(out=bde_r[:], in_=bde_d)
            if use_ln:
                lnw_r = con.tile([1, H], f32)
                nc.sync.dma_start(out=lnw_r[:], in_=lnw_d)
                lnb_r = con.tile([1, H], f32)
                nc.sync.dma_start(out=lnb_r[:], in_=lnb_d)

            # warm the ACT table set (Ln+Exp live in natural_log_exp_and_others)
            warm = wk.tile([1, 1], f32)
            nc.vector.memset(warm[:], 1.0)
            warm2 = wk.tile([1, 1], f32)
            nc.scalar.activation(warm2[:], warm[:], Act.Ln)
            nc.scalar.activation(warm2[:], warm2[:], Act.Exp)

            # --- xT [H, L] via PE transpose ---
            xT_ps = ppb.tile([H, L], f32, tag="big")
            nc.tensor.transpose(xT_ps[:, 0:128], xc[0][:], ident[:])
            nc.tensor.transpose(xT_ps[:, 128:160], xc[1][:], ident[0:32, 0:32])
            xT = wk.tile([H, L], f32)
            nc.scalar.copy(xT[:], xT_ps[:])

            # --- sampler^2 transposed: s2T = Square(invH4 * (Wq @ xT) + bq*invH4)
            qT_ps = ppb.tile([H, L], f32, tag="big")
            nc.tensor.matmul(qT_ps[:], wqt[:], xT[:], start=True, stop=True)
            s2T = wk.tile([H, L], f32)
            nc.scalar.activation(s2T[:], qT_ps[:], Act.Square, bias=bqp[:], scale=inv_h4)

            # --- K chunks [p, L] ---
            Kc = []
            for off, p in CHUNKS:
                kps = ppb.tile([p, L], f32, tag="big")
                nc.tensor.matmul(kps[:], s2T[:, off : off + p], s2T[:], start=True, stop=True)
                k_sb = wk.tile([p, L], f32, tag=f"K{off}")
                nc.scalar.copy(k_sb[:], kps[:])
                Kc.append(k_sb)

            # --- d = diag(K): d_row via ones-matmul over s4T = s2T*s2T ---
            s4T = wk.tile([H, L], f32)
            nc.vector.tensor_mul(s4T[:], s2T[:], s2T[:])
            drow_ps = pps.tile([1, L], f32, tag="small")
            nc.tensor.matmul(drow_ps[:], ones64[:], s4T[:], start=True, stop=True)
            drow = wk.tile([1, L], f32)
            nc.vector.tensor_copy(drow[:], drow_ps[:])
            dcol_ps = []
            for off, p in CHUNKS:
                dps = pps.tile([p, 1], f32, tag="small")
                nc.tensor.matmul(dps[:], s4T[:, off : off + p], ones64[:], start=True, stop=True)
                dcol_ps.append(dps)

            # S0 = sum_k w_k d_k  (scalar [1,1])
            S0_t = wk.tile([1, 1], f32)
            if use_w:
                wd_row = wk.tile([1, L], f32)
                nc.vector.tensor_mul(wd_row[:], drow[:], wrow[:])
                nc.vector.reduce_sum(S0_t[:], wd_row[:], axis=mybir.AxisListType.X)
            else:
                nc.vector.reduce_sum(S0_t[:], drow[:], axis=mybir.AxisListType.X)
            # scale*S0 and -scale*S0 as [1,1]
            S0s_t = wk.tile([1, 1], f32)
            nc.vector.tensor_scalar(S0s_t[:], S0_t[:], scale, None, op0=Alu.mult)
            nS0s_t = wk.tile([1, 1], f32)
            nc.vector.tensor_scalar(nS0s_t[:], S0_t[:], -scale, None, op0=Alu.mult)
            # broadcast scale*S0 to partition columns per chunk
            onesr = con.tile([1, 128], f32)
            nc.vector.memset(onesr[:], 1.0)
            S0scol = []
            for off, p in CHUNKS:
                sps = pps.tile([p, 1], f32, tag="small")
                nc.tensor.matmul(sps[:], onesr[0:1, 0:p], S0s_t[:], start=True, stop=True)
                scol = wk.tile([p, 1], f32, tag=f"s0c{off}")
                nc.vector.tensor_copy(scol[:], sps[:])
                S0scol.append(scol)

            # w as per-partition columns (for K diag(w) and a-row weighting)
            wcol = [None, None]
            if use_w:
                for i, (off, p) in enumerate(CHUNKS):
                    wps = pps.tile([p, 1], f32, tag="small")
                    nc.tensor.transpose(wps[:], wrow[0:1, off : off + p], ident[0:1, 0:1])
                    wc = wk.tile([p, 1], f32, tag=f"wc{off}")
                    nc.vector.tensor_copy(wc[:], wps[:])
                    wcol[i] = wc

            # --- KK (optionally weighted) for a = diag(K diag(w) K) ---
            KKc = []
            for i, (off, p) in enumerate(CHUNKS):
                kk = wk.tile([p, L], f32, tag=f"KK{off}")
                nc.vector.tensor_mul(kk[:], Kc[i][:], Kc[i][:])
                if use_w:
                    nc.vector.tensor_scalar(kk[:], kk[:], wcol[i][:], None, op0=Alu.mult)
                KKc.append(kk)
            arow_ps = pps.tile([1, L], f32, tag="small")
            nc.tensor.matmul(arow_ps[:], ones128[:], KKc[0][:], start=True, stop=False)
            nc.tensor.matmul(arow_ps[:], ones128[0:32, :], KKc[1][:], start=False, stop=True)
            arow = wk.tile([1, L], f32)
            nc.vector.tensor_copy(arow[:], arow_ps[:])

            # --- rank-2 factors: v0 = -S0s*d + s*a ; v1 = s*d ---
            v0_r = wk.tile([1, L], f32)
            tmp_r = wk.tile([1, L], f32)
            nc.vector.tensor_scalar(tmp_r[:], arow[:], scale, None, op0=Alu.mult)
            nc.vector.scalar_tensor_tensor(
                v0_r[:], drow[:], nS0s_t[:], tmp_r[:], op0=Alu.mult, op1=Alu.add
            )
            v1_r = wk.tile([1, L], f32)
            nc.vector.tensor_scalar(v1_r[:], drow[:], scale, None, op0=Alu.mult)

            # --- A2s = -2*scale * K diag(w) K  (PSUM, per chunk) ---
            wK2s = []
            for i, (off, p) in enumerate(CHUNKS):
                t = wk.tile([p, L], f32, tag=f"wk2{off}")
                if use_w:
                    nc.vector.tensor_scalar(
                        t[:], Kc[i][:], wcol[i][:], -2.0 * scale, op0=Alu.mult, op1=Alu.mult
                    )
                else:
                    nc.vector.tensor_scalar(t[:], Kc[i][:], -2.0 * scale, None, op0=Alu.mult)
                wK2s.append(t)

            # diag contribution: dsel[p, f] = -scale*d_p at f == p+off else 0
            dsel = []
            for i, (off, p) in enumerate(CHUNKS):
                dsc = wk.tile([p, 1], f32, tag=f"dsc{off}")
                nc.vector.tensor_scalar(dsc[:], dcol_ps[i][:], -scale, None, op0=Alu.mult)
                ds = wk.tile([p, L], f32, tag=f"dsel{off}")
                nc.gpsimd.affine_select(
                    out=ds[:],
                    in_=dsc[:, 0:1].broadcast_to([p, L]),
                    compare_op=Alu.is_equal,
                    fill=0.0,
                    base=-off,
                    pattern=[[1, L]],
                    channel_multiplier=-1,
                )
                dsel.append(ds)

            # --- score + exp per chunk (transposed orientation) ---
            ec = []
            for i, (off, p) in enumerate(CHUNKS):
                a2s = ppb.tile([p, L], f32, tag="big")
                nc.tensor.matmul(a2s[:], Kc[0][:, off : off + p], wK2s[0][:], start=True, stop=False)
                nc.tensor.matmul(a2s[:], Kc[1][:, off : off + p], wK2s[1][:], start=False, stop=True)
                rs = ppb.tile([p, L], f32, tag="big")
                nc.tensor.matmul(rs[:], drow[0:1, off : off + p], v0_r[:], start=True, stop=False)
                nc.tensor.matmul(rs[:], arow[0:1, off : off + p], v1_r[:], start=False, stop=True)

                t1 = wk.tile([p, L], f32, tag=f"t1{off}")
                # t1 = K*(S0*scale) + A2s
                nc.vector.scalar_tensor_tensor(
                    t1[:], Kc[i][:], S0scol[i][:], a2s[:], op0=Alu.mult, op1=Alu.add
                )
                # t1 = t1 * K   (= -scale*(2A - S0*K) ⊙ K)
                nc.vector.tensor_mul(t1[:], t1[:], Kc[i][:])
                # t1 += Rs  (rank-2 part)
                nc.vector.tensor_add(t1[:], t1[:], rs[:])
                # t1 += diag
                nc.vector.tensor_add(t1[:], t1[:], dsel[i][:])
                if use_mask:
                    nc.vector.tensor_add(t1[:], t1[:], masktc[i][:])
                e = wk.tile([p, L], f32, tag=f"e{off}")
                nc.scalar.activation(e[:], t1[:], Act.Exp)
                ec.append(e)

            # --- value projection with appended ones column ---
            Vh = []
            for i, (off, p) in enumerate(CHUNKS):
                vps = ppm.tile([p, H], f32, tag="p64")
                nc.tensor.matmul(vps[:], xT[:, off : off + p], wvt[:], start=True, stop=True)
                vh = wk.tile([p, H + 1], f32, tag=f"vh{off}")
                nc.vector.tensor_copy(vh[:, 0:H], vps[:])
                nc.vector.memset(vh[:, H : H + 1], 1.0)
                Vh.append(vh)

            # ctxT' [H+1, L]: rows 0..H-1 = V^T e^T, row H = Z (softmax denominators)
            ctxT_ps = ppb.tile([H + 1, L], f32, tag="big")
            nc.tensor.matmul(ctxT_ps[:], Vh[0][:], ec[0][:], start=True, stop=False)
            nc.tensor.matmul(ctxT_ps[:], Vh[1][:], ec[1][:], start=False, stop=True)
            ctxT = wk.tile([H + 1, L], f32)
            nc.scalar.copy(ctxT[:], ctxT_ps[:])

            # output projection (still transposed): outT = Wd @ ctxT
            outT_ps = ppb.tile([H, L], f32, tag="big")
            nc.tensor.matmul(outT_ps[:], wdt[:], ctxT[0:H, :], start=True, stop=True)
            outT = wk.tile([H, L], f32)
            nc.scalar.copy(outT[:], outT_ps[:])

            # ln weight/bias broadcast tiles if needed
            if use_bde:
                bde_ps = ppm.tile([128, H], f32, tag="p64")
                nc.tensor.matmul(bde_ps[:], onesr[:], bde_r[:], start=True, stop=True)
                bde_b = wk.tile([128, H], f32)
                nc.vector.tensor_copy(bde_b[:], bde_ps[:])
            if use_ln:
                lnw_ps = ppm.tile([128, H], f32, tag="p64")
                nc.tensor.matmul(lnw_ps[:], onesr[:], lnw_r[:], start=True, stop=True)
                lnw_b = wk.tile([128, H], f32)
                nc.vector.tensor_copy(lnw_b[:], lnw_ps[:])
                lnb_ps = ppm.tile([128, H], f32, tag="p64")
                nc.tensor.matmul(lnb_ps[:], onesr[:], lnb_r[:], start=True, stop=True)
                lnb_b = wk.tile([128, H], f32)
                nc.vector.tensor_copy(lnb_b[:], lnb_ps[:])

            # --- per chunk: back to natural layout, normalize, residual, LN ---
            for i, (off, p) in enumerate(CHUNKS):
                zps = pps.tile([p, 1], f32, tag="small")
                # identity slice picked at base partition H so operand base
                # partitions match (PE requirement); ident[H,H] == 1.
                nc.tensor.transpose(zps[:], ctxT[H : H + 1, off : off + p], ident[H : H + 1, H : H + 1])
                rcol = wk.tile([p, 1], f32, tag=f"rc{off}")
                nc.vector.reciprocal(rcol[:], zps[:])

                ops = ppm.tile([p, H], f32, tag="p64")
                nc.tensor.transpose(ops[:], outT[:, off : off + p], ident[0:H, 0:H])

                res = wk.tile([p, H], f32, tag=f"res{off}")
                # res = out_nat * r + x
                nc.vector.scalar_tensor_tensor(
                    res[:], ops[:], rcol[:], xc[i][:], op0=Alu.mult, op1=Alu.add
                )
                if use_bde:
                    nc.vector.tensor_add(res[:], res[:], bde_b[0:p, :])

                stats = wk.tile([p, 6], f32, tag=f"st{off}")
                nc.vector.bn_stats(stats[:], res[:])
                mv = wk.tile([p, 2], f32, tag=f"mv{off}")
                nc.vector.bn_aggr(mv[:], stats[:])
                # rstd = exp(-0.5*ln(var+eps)); Ln/Exp share one ACT table set
                lnv = wk.tile([p, 1], f32, tag=f"lnv{off}")
                nc.scalar.activation(lnv[:], mv[:, 1:2], Act.Ln, bias=epsc[0:p, :])
                rstd = wk.tile([p, 1], f32, tag=f"rst{off}")
                nc.scalar.activation(rstd[:], lnv[:], Act.Exp, scale=-0.5)

                y_t = wk.tile([p, H], f32, tag=f"y{off}")
                nc.vector.tensor_scalar(
                    y_t[:], res[:], mv[:, 0:1], rstd[:], op0=Alu.subtract, op1=Alu.mult
                )
                if use_ln:
                    nc.vector.tensor_mul(y_t[:], y_t[:], lnw_b[0:p, :])
                    nc.vector.tensor_add(y_t[:], y_t[:], lnb_b[0:p, :])
                nc.sync.dma_start(out=y_d[off : off + p, :], in_=y_t[:])

    nc.compile()
    return nc


def _prepare(inputs):
    x = np.ascontiguousarray(np.asarray(inputs["input_tensor"], dtype=np.float32))
    mask = np.ascontiguousarray(np.asarray(inputs["attention_mask"], dtype=np.float32))
    Wq = np.asarray(inputs["Wq"], dtype=np.float32)
    bq = np.asarray(inputs["bq"], dtype=np.float32)
    Wv = np.asarray(inputs["Wv"], dtype=np.float32)
    bv = np.asarray(inputs["bv"], dtype=np.float32)
    Wd = np.asarray(inputs["Wd"], dtype=np.float32)
    bd = np.asarray(inputs["bd"], dtype=np.float32)
    ln_w = np.asarray(inputs["ln_w"], dtype=np.float32)
    ln_b = np.asarray(inputs["ln_b"], dtype=np.float32)
    scale = np.float32(np.asarray(inputs["scale_factor"]).reshape(()))

    use_mask = bool(np.any(mask != 0.0))
    wvals = (mask[:, 0, :] > -10000.0).astype(np.float32)
    use_w = not bool(np.all(wvals == 1.0))
    bde = bd + Wd @ bv  # value bias folded through the output projection
    use_bde = bool(np.any(bde != 0.0))
    use_ln = not (bool(np.all(ln_w == 1.0)) and bool(np.all(ln_b == 0.0)))

    flags = (use_mask, use_w, use_bde, use_ln, float(scale))
    shared = {
        "wqt": np.ascontiguousarray(Wq.T),
        "wvt": np.ascontiguousarray(Wv.T),
        "wdt": np.ascontiguousarray(Wd.T),
        "bqp": np.ascontiguousarray((bq * (H ** -0.25)).reshape(H, 1)),
    }
    if use_bde:
        shared["bde"] = np.ascontiguousarray(bde.reshape(1, H))
    if use_ln:
        shared["lnw"] = np.ascontiguousarray(ln_w.reshape(1, H))
        shared["lnb"] = np.ascontiguousarray(ln_b.reshape(1, H))

    in_maps = []
    for c in range(N_CORES):
        m = dict(shared)
        m["x"] = np.ascontiguousarray(x[c])
        if use_mask:
            m["maskt"] = np.ascontiguousarray(mask[c].T)
        if use_w:
            m["wrow"] = np.ascontiguousarray(wvals[c].reshape(1, L))
        in_maps.append(m)
    return flags, in_maps


def _get_program(flags):
    if flags not in _programs:
        _programs[flags] = _build_program(*flags)
    return _programs[flags]


def kernel(**inputs):
    from concourse.bass_utils import run_bass_kernel_spmd

    flags, in_maps = _prepare(inputs)
    nc = _get_program(flags)
    res = run_bass_kernel_spmd(nc, in_maps, core_ids=list(range(N_CORES)))
    out = np.stack([res.results[c]["y"] for c in range(N_CORES)], axis=0)
    return out.astype(np.float32)


# revision 28
# speedup vs baseline: 1.3260x; 1.3260x over previous
"""DPP attention-3 Trainium2 kernel.

Data-parallel across 8 NeuronCores: one batch element per core; all
weights replicated.  The reference's [B,L,L,L] det_values tensor is never
materialized: since K = s2 @ s2.T is exactly symmetric, the k-reduction of
the 3x3 determinants collapses to

    marginal[i,j] = S0*(d_i d_j - K_ij^2) - d_i a_j - a_i d_j + 2 K_ij A_ij

with  A = K diag(w) K,  a = diag(A),  d = diag(K),  S0 = sum_k w_k d_k.

Everything is computed in the transposed [j, i] orientation (score is
symmetric up to the additive mask, which is fed pre-transposed from the
host), so the exp'd scores are directly usable as the stationary operand
of the context matmul and the row-sums Z fall out of an extra all-ones
column appended to the value matrix.

Matmul operands are kept in bf16 (fp32 PSUM accumulation): fp32 matmuls
cost 4 cycles per output row on TRN2 vs 1 for bf16, and the score scale
here (|score| < 1) makes the bf16 rounding negligible (~6e-5 final rel
err measured).  The residual/LayerNorm path stays fp32.
"""

import numpy as np

B, L, H = 8, 160, 64
N_CORES = 8
EPS = 1e-12
CHUNKS = [(0, 128), (128, 32)]  # partition chunks covering L=160

_programs = {}  # (flags..., scale) -> nc


def _build_program(use_mask, use_w, use_bde, use_ln, scale):
    import concourse.bass as bass
    import concourse.bacc as bacc_mod
    import concourse.tile as tile
    from concourse import bacc, mybir
    from concourse.masks import make_identity

    f32 = mybir.dt.float32
    bf16 = mybir.dt.bfloat16
    Alu = mybir.AluOpType
    Act = mybir.ActivationFunctionType

    nc = bacc.Bacc(
        "TRN2",
        target_bir_lowering=False,
        debug=False,
        enable_asserts=False,
        num_devices=N_CORES,
    )

    inv_h4 = float(H ** -0.25)

    x_d = nc.dram_tensor("x", [L, H], f32, kind="ExternalInput").ap()
    wqt_d = nc.dram_tensor("wqt", [H, H], bf16, kind="ExternalInput").ap()
    wvt_d = nc.dram_tensor("wvt", [H, H], bf16, kind="ExternalInput").ap()
    wdt_d = nc.dram_tensor("wdt", [H, H], bf16, kind="ExternalInput").ap()
    bqp_d = nc.dram_tensor("bqp", [H, 1], f32, kind="ExternalInput").ap()
    maskt_d = wrow_d = bde_d = lnw_d = lnb_d = None
    if use_mask:
        maskt_d = nc.dram_tensor("maskt", [L, L], f32, kind="ExternalInput").ap()
    if use_w:
        wrow_d = nc.dram_tensor("wrow", [1, L], f32, kind="ExternalInput").ap()
    if use_bde:
        bde_d = nc.dram_tensor("bde", [1, H], f32, kind="ExternalInput").ap()
    if use_ln:
        lnw_d = nc.dram_tensor("lnw", [1, H], f32, kind="ExternalInput").ap()
        lnb_d = nc.dram_tensor("lnb", [1, H], f32, kind="ExternalInput").ap()
    y_d = nc.dram_tensor("y", [L, H], f32, kind="ExternalOutput").ap()

    with tile.TileContext(nc) as tc:
        from contextlib import ExitStack

        with ExitStack() as ctx:
            con = ctx.enter_context(tc.tile_pool(name="con", bufs=1))
            wk = ctx.enter_context(tc.tile_pool(name="wk", bufs=1))
            ppb = ctx.enter_context(tc.tile_pool(name="ppb", bufs=4, space="PSUM"))
            pps = ctx.enter_context(tc.tile_pool(name="pps", bufs=2, space="PSUM"))
            ppm = ctx.enter_context(tc.tile_pool(name="ppm", bufs=2, space="PSUM"))

            # --- constants ---
            ident = con.tile([128, 128], f32)
            make_identity(nc, ident[:])
            ones64b = con.tile([H, 1], bf16)
            nc.gpsimd.memset(ones64b[:], 1.0)
            ones128b = con.tile([128, 1], bf16)
            nc.gpsimd.memset(ones128b[:], 1.0)
            onesr = con.tile([1, 128], f32)
            nc.gpsimd.memset(onesr[:], 1.0)
            epsc = con.tile([128, 1], f32)
            nc.gpsimd.memset(epsc[:], EPS)

            wqt = con.tile([H, H], bf16)
            nc.sync.dma_start(out=wqt[:], in_=wqt_d)
            wvt = con.tile([H, H], bf16)
            nc.sync.dma_start(out=wvt[:], in_=wvt_d)
            wdt = con.tile([H, H], bf16)
            nc.sync.dma_start(out=wdt[:], in_=wdt_d)
            bqp = con.tile([H, 1], f32)
            nc.sync.dma_start(out=bqp[:], in_=bqp_d)

            xc = []
            for off, p in CHUNKS:
                t = con.tile([p, H], f32, tag=f"x{off}")
                nc.sync.dma_start(out=t[:], in_=x_d[off : off + p, :])
                xc.append(t)
            masktc = []
            if use_mask:
                for off, p in CHUNKS:
                    t = con.tile([p, L], f32, tag=f"mt{off}")
                    nc.sync.dma_start(out=t[:], in_=maskt_d[off : off + p, :])
                    masktc.append(t)
            if use_w:
                wrow = con.tile([1, L], f32)
                nc.sync.dma_start(out=wrow[:], in_=wrow_d)
            if use_bde:
                bde_r = con.tile([1, H], f32)
                nc.sync.dma_start(out=bde_r[:], in_=bde_d)
            if use_ln:
                lnw_r = con.tile([1, H], f32)
                nc.sync.dma_start(out=lnw_r[:], in_=lnw_d)
                lnb_r = con.tile([1, H], f32)
                nc.sync.dma_start(out=lnb_r[:], in_=lnb_d)

            # pull the single ACT table load off the critical path
            warm = wk.tile([1, 1], f32)
            nc.gpsimd.memset(warm[:], 1.0)
            warm2 = wk.tile([1, 1], f32)
            nc.scalar.copy(warm2[:], warm[:])

            # --- xT [H, L] via PE transpose (fp32 in PSUM, cast to bf16) ---
            xT_ps = ppb.tile([H, L], f32, tag="big")
            nc.tensor.transpose(xT_ps[:, 0:128], xc[0][:], ident[:])
            nc.tensor.transpose(xT_ps[:, 128:160], xc[1][:], ident[0:32, 0:32])
            xT = wk.tile([H, L], bf16)
            nc.vector.tensor_copy(xT[:], xT_ps[:])

            # --- sampler^2 transposed: s2T = Square(invH4 * (Wq @ xT) + bq*invH4)
            qT_ps = ppb.tile([H, L], f32, tag="big")
            nc.tensor.matmul(qT_ps[:], wqt[:], xT[:], start=True, stop=True)
            s2T = wk.tile([H, L], bf16)
            nc.scalar.activation(s2T[:], qT_ps[:], Act.Square, bias=bqp[:], scale=inv_h4)

            # --- K chunks [p, L] ---
            Kc = []
            for i, (off, p) in enumerate(CHUNKS):
                kps = ppb.tile([p, L], f32, tag="big")
                nc.tensor.matmul(kps[:], s2T[:, off : off + p], s2T[:], start=True, stop=True)
                k_sb = wk.tile([p, L], bf16, tag=f"K{off}")
                if i == 0:
                    nc.scalar.copy(k_sb[:], kps[:])
                else:
                    nc.vector.tensor_copy(k_sb[:], kps[:])
                Kc.append(k_sb)

            # --- d = diag(K) via s4T = s2T*s2T ---
            s4T = wk.tile([H, L], bf16)
            nc.vector.tensor_mul(s4T[:], s2T[:], s2T[:])
            drow_ps = pps.tile([1, L], f32, tag="small")
            nc.tensor.matmul(drow_ps[:], ones64b[:], s4T[:], start=True, stop=True)
            drow = wk.tile([1, L], bf16)
            nc.vector.tensor_copy(drow[:], drow_ps[:])
            dcol_ps = []
            for off, p in CHUNKS:
                dps = pps.tile([p, 1], f32, tag="small")
                nc.tensor.matmul(dps[:], s4T[:, off : off + p], ones64b[:], start=True, stop=True)
                dcol_ps.append(dps)

            # S0 = sum_k w_k d_k  (scalar [1,1], fp32)
            S0_t = wk.tile([1, 1], f32)
            if use_w:
                wd_row = wk.tile([1, L], f32)
                nc.vector.tensor_mul(wd_row[:], drow[:], wrow[:])
                nc.vector.reduce_sum(S0_t[:], wd_row[:], axis=mybir.AxisListType.X)
            else:
                nc.vector.reduce_sum(S0_t[:], drow[:], axis=mybir.AxisListType.X)
            S0s_t = wk.tile([1, 1], f32)
            nc.vector.tensor_scalar(S0s_t[:], S0_t[:], scale, None, op0=Alu.mult)
            nS0s_t = wk.tile([1, 1], f32)
            nc.vector.tensor_scalar(nS0s_t[:], S0_t[:], -scale, None, op0=Alu.mult)
            # broadcast scale*S0 down partition columns per chunk
            S0scol = []
            for off, p in CHUNKS:
                sps = pps.tile([p, 1], f32, tag="small")
                nc.tensor.matmul(sps[:], onesr[0:1, 0:p], S0s_t[:], start=True, stop=True)
                scol = wk.tile([p, 1], f32, tag=f"s0c{off}")
                nc.vector.tensor_copy(scol[:], sps[:])
                S0scol.append(scol)

            wcol = [None, None]
            if use_w:
                for i, (off, p) in enumerate(CHUNKS):
                    wps = pps.tile([p, 1], f32, tag="small")
                    nc.tensor.transpose(wps[:], wrow[0:1, off : off + p], ident[0:1, 0:1])
                    wc = wk.tile([p, 1], f32, tag=f"wc{off}")
                    nc.vector.tensor_copy(wc[:], wps[:])
                    wcol[i] = wc

            # --- KK (optionally weighted) for a = diag(K diag(w) K) ---
            KKc = []
            for i, (off, p) in enumerate(CHUNKS):
                kk = wk.tile([p, L], bf16, tag=f"KK{off}")
                nc.vector.tensor_mul(kk[:], Kc[i][:], Kc[i][:])
                if use_w:
                    nc.vector.tensor_scalar(kk[:], kk[:], wcol[i][:], None, op0=Alu.mult)
                KKc.append(kk)
            arow_ps = pps.tile([1, L], f32, tag="small")
            nc.tensor.matmul(arow_ps[:], ones128b[:], KKc[0][:], start=True, stop=False)
            nc.tensor.matmul(arow_ps[:], ones128b[0:32, :], KKc[1][:], start=False, stop=True)
            arow = wk.tile([1, L], bf16)
            nc.vector.tensor_copy(arow[:], arow_ps[:])

            # --- rank-2 factors: v0 = -S0s*d + s*a ; v1 = s*d  (bf16 rows) ---
            v0_r = wk.tile([1, L], bf16)
            tmp_r = wk.tile([1, L], bf16)
            nc.vector.tensor_scalar(tmp_r[:], arow[:], scale, None, op0=Alu.mult)
            nc.vector.scalar_tensor_tensor(
                v0_r[:], drow[:], nS0s_t[:], tmp_r[:], op0=Alu.mult, op1=Alu.add
            )
            v1_r = wk.tile([1, L], bf16)
            nc.vector.tensor_scalar(v1_r[:], drow[:], scale, None, op0=Alu.mult)

            # --- A2s = -2*scale * K diag(w) K  (PSUM, per chunk) ---
            wK2s = []
            for i, (off, p) in enumerate(CHUNKS):
                t = wk.tile([p, L], bf16, tag=f"wk2{off}")
                if use_w:
                    nc.vector.tensor_scalar(
                        t[:], Kc[i][:], wcol[i][:], -2.0 * scale, op0=Alu.mult, op1=Alu.mult
                    )
                else:
                    nc.vector.tensor_scalar(t[:], Kc[i][:], -2.0 * scale, None, op0=Alu.mult)
                wK2s.append(t)

            # diag contribution: dsel[p, f] = -scale*d_p at f == p+off else 0
            dsel = []
            for i, (off, p) in enumerate(CHUNKS):
                dsc = wk.tile([p, 1], f32, tag=f"dsc{off}")
                nc.vector.tensor_scalar(dsc[:], dcol_ps[i][:], -scale, None, op0=Alu.mult)
                ds = wk.tile([p, L], f32, tag=f"dsel{off}")
                nc.gpsimd.affine_select(
                    out=ds[:],
                    in_=dsc[:, 0:1].broadcast_to([p, L]),
                    compare_op=Alu.is_equal,
                    fill=0.0,
                    base=-off,
                    pattern=[[1, L]],
                    channel_multiplier=-1,
                )
                dsel.append(ds)

            # --- score + exp per chunk (transposed orientation) ---
            ec = []
            for i, (off, p) in enumerate(CHUNKS):
                a2s = ppb.tile([p, L], f32, tag="big")
                nc.tensor.matmul(a2s[:], Kc[0][:, off : off + p], wK2s[0][:], start=True, stop=False)
                nc.tensor.matmul(a2s[:], Kc[1][:, off : off + p], wK2s[1][:], start=False, stop=True)
                rs = ppb.tile([p, L], f32, tag="big")
                nc.tensor.matmul(rs[:], drow[0:1, off : off + p], v0_r[:], start=True, stop=False)
                nc.tensor.matmul(rs[:], arow[0:1, off : off + p], v1_r[:], start=False, stop=True)

                t1 = wk.tile([p, L], f32, tag=f"t1{off}")
                # t1 = K*(S0*scale) + A2s
                nc.vector.scalar_tensor_tensor(
                    t1[:], Kc[i][:], S0scol[i][:], a2s[:], op0=Alu.mult, op1=Alu.add
                )
                # t1 = t1 * K   (= -scale*(2A - S0*K) ⊙ K)
                nc.vector.tensor_mul(t1[:], t1[:], Kc[i][:])
                # t1 += Rs  (rank-2 part)
                nc.vector.tensor_add(t1[:], t1[:], rs[:])
                # t1 += diag
                nc.vector.tensor_add(t1[:], t1[:], dsel[i][:])
                if use_mask:
                    nc.vector.tensor_add(t1[:], t1[:], masktc[i][:])
                e = wk.tile([p, L], bf16, tag=f"e{off}")
                nc.scalar.activation(e[:], t1[:], Act.Exp)
                ec.append(e)

            # --- value projection (bf16) ---
            Vh = []
            for i, (off, p) in enumerate(CHUNKS):
                vps = ppm.tile([p, H], f32, tag="p64")
                nc.tensor.matmul(vps[:], xT[:, off : off + p], wvt[:], start=True, stop=True)
                vh = wk.tile([p, H], bf16, tag=f"vh{off}")
                nc.vector.tensor_copy(vh[:], vps[:])
                Vh.append(vh)

            # softmax denominators Z (fp32 PSUM accumulation of bf16 e)
            zcol_ps = []
            for off, p in CHUNKS:
                zps = pps.tile([p, 1], f32, tag="small")
                nc.tensor.matmul(zps[:], ec[0][:, off : off + p], ones128b[:], start=True, stop=False)
                nc.tensor.matmul(zps[:], ec[1][:, off : off + p], ones128b[0:32, :], start=False, stop=True)
                zcol_ps.append(zps)

            # ctxT [H, L] = V^T e^T
            ctxT_ps = ppb.tile([H, L], f32, tag="big")
            nc.tensor.matmul(ctxT_ps[:], Vh[0][:], ec[0][:], start=True, stop=False)
            nc.tensor.matmul(ctxT_ps[:], Vh[1][:], ec[1][:], start=False, stop=True)
            ctxT = wk.tile([H, L], bf16)
            nc.scalar.copy(ctxT[:], ctxT_ps[:])

            # output projection (still transposed): outT = Wd @ ctxT
            outT_ps = ppb.tile([H, L], f32, tag="big")
            nc.tensor.matmul(outT_ps[:], wdt[:], ctxT[:], start=True, stop=True)
            outT = wk.tile([H, L], f32)
            nc.scalar.copy(outT[:], outT_ps[:])

            if use_bde:
                bde_ps = ppm.tile([128, H], f32, tag="p64")
                nc.tensor.matmul(bde_ps[:], onesr[:], bde_r[:], start=True, stop=True)
                bde_b = wk.tile([128, H], f32)
                nc.vector.tensor_copy(bde_b[:], bde_ps[:])
            if use_ln:
                lnw_ps = ppm.tile([128, H], f32, tag="p64")
                nc.tensor.matmul(lnw_ps[:], onesr[:], lnw_r[:], start=True, stop=True)
                lnw_b = wk.tile([128, H], f32)
                nc.vector.tensor_copy(lnw_b[:], lnw_ps[:])
                lnb_ps = ppm.tile([128, H], f32, tag="p64")
                nc.tensor.matmul(lnb_ps[:], onesr[:], lnb_r[:], start=True, stop=True)
                lnb_b = wk.tile([128, H], f32)
                nc.vector.tensor_copy(lnb_b[:], lnb_ps[:])

            # --- per chunk: natural layout, normalize, residual, LayerNorm ---
            for i, (off, p) in enumerate(CHUNKS):
                rcol = wk.tile([p, 1], f32, tag=f"rc{off}")
                nc.vector.reciprocal(rcol[:], zcol_ps[i][:])

                ops = ppm.tile([p, H], f32, tag="p64")
                nc.tensor.transpose(ops[:], outT[:, off : off + p], ident[0:H, 0:H])

                res = wk.tile([p, H], f32, tag=f"res{off}")
                # res = out_nat * r + x
                nc.vector.scalar_tensor_tensor(
                    res[:], ops[:], rcol[:], xc[i][:], op0=Alu.mult, op1=Alu.add
                )
                if use_bde:
                    nc.vector.tensor_add(res[:], res[:], bde_b[0:p, :])

                stats = wk.tile([p, 6], f32, tag=f"st{off}")
                nc.vector.bn_stats(stats[:], res[:])
                mv = wk.tile([p, 2], f32, tag=f"mv{off}")
                nc.vector.bn_aggr(mv[:], stats[:])
                # rstd = exp(-0.5*ln(var+eps)); Ln+Exp live in one table set
                lnv = wk.tile([p, 1], f32, tag=f"lnv{off}")
                nc.scalar.activation(lnv[:], mv[:, 1:2], Act.Ln, bias=epsc[0:p, :])
                rstd = wk.tile([p, 1], f32, tag=f"rst{off}")
                nc.scalar.activation(rstd[:], lnv[:], Act.Exp, scale=-0.5)

                y_t = wk.tile([p, H], f32, tag=f"y{off}")
                nc.vector.tensor_scalar(
                    y_t[:], res[:], mv[:, 0:1], rstd[:], op0=Alu.subtract, op1=Alu.mult
                )
                if use_ln:
                    nc.vector.tensor_mul(y_t[:], y_t[:], lnw_b[0:p, :])
                    nc.vector.tensor_add(y_t[:], y_t[:], lnb_b[0:p, :])
                nc.sync.dma_start(out=y_d[off : off + p, :], in_=y_t[:])

    # Compile with the combined Ln+Exp activation-table set preferred, so a
    # single ACT_TABLE_LOAD covers Square/Copy/Exp/Ln (the default greedy
    # selection alternates between the exp-only and ln-only sets: 6 loads,
    # ~7.7us of ACT time).  The set *order* must be preserved — the position
    # in this dict is the act_func_set_id walrus resolves against
    # act_info.json — so instead of reordering, hide this kernel's functions
    # from every other set, forcing the selector onto the combined one at
    # its true index.
    orig_tables = bacc_mod.get_activation_tables
    mine = {Act.Exp, Act.Ln, Act.Square, Act.Copy, Act.Identity}

    def _patched(arch):
        tabs = orig_tables(arch)
        assert "natural_log_exp_and_others" in tabs
        return {
            n: (fs if n == "natural_log_exp_and_others" else fs - mine)
            for n, fs in tabs.items()
        }

    bacc_mod.get_activation_tables = _patched
    try:
        nc.compile()
    finally:
        bacc_mod.get_activation_tables = orig_tables
    return nc


def _prepare(inputs):
    import ml_dtypes

    bf = ml_dtypes.bfloat16
    x = np.ascontiguousarray(np.asarray(inputs["input_tensor"], dtype=np.float32))
    mask = np.ascontiguousarray(np.asarray(inputs["attention_mask"], dtype=np.float32))
    Wq = np.asarray(inputs["Wq"], dtype=np.float32)
    bq = np.asarray(inputs["bq"], dtype=np.float32)
    Wv = np.asarray(inputs["Wv"], dtype=np.float32)
    bv = np.asarray(inputs["bv"], dtype=np.float32)
    Wd = np.asarray(inputs["Wd"], dtype=np.float32)
    bd = np.asarray(inputs["bd"], dtype=np.float32)
    ln_w = np.asarray(inputs["ln_w"], dtype=np.float32)
    ln_b = np.asarray(inputs["ln_b"], dtype=np.float32)
    scale = np.float32(np.asarray(inputs["scale_factor"]).reshape(()))

    use_mask = bool(np.any(mask != 0.0))
    wvals = (mask[:, 0, :] > -10000.0).astype(np.float32)
    use_w = not bool(np.all(wvals == 1.0))
    bde = bd + Wd @ bv  # value bias folded through the output projection
    use_bde = bool(np.any(bde != 0.0))
    use_ln = not (bool(np.all(ln_w == 1.0)) and bool(np.all(ln_b == 0.0)))

    flags = (use_mask, use_w, use_bde, use_ln, float(scale))
    shared = {
        "wqt": np.ascontiguousarray(Wq.T).astype(bf),
        "wvt": np.ascontiguousarray(Wv.T).astype(bf),
        "wdt": np.ascontiguousarray(Wd.T).astype(bf),
        "bqp": np.ascontiguousarray((bq * (H ** -0.25)).reshape(H, 1)),
    }
    if use_bde:
        shared["bde"] = np.ascontiguousarray(bde.reshape(1, H))
    if use_ln:
        shared["lnw"] = np.ascontiguousarray(ln_w.reshape(1, H))
        shared["lnb"] = np.ascontiguousarray(ln_b.reshape(1, H))

    in_maps = []
    for c in range(N_CORES):
        m = dict(shared)
        m["x"] = np.ascontiguousarray(x[c])
        if use_mask:
            m["maskt"] = np.ascontiguousarray(mask[c].T)
        if use_w:
            m["wrow"] = np.ascontiguousarray(wvals[c].reshape(1, L))
        in_maps.append(m)
    return flags, in_maps


def _get_program(flags):
    if flags not in _programs:
        _programs[flags] = _build_program(*flags)
    return _programs[flags]


def kernel(**inputs):
    from concourse.bass_utils import run_bass_kernel_spmd

    flags, in_maps = _prepare(inputs)
    nc = _get_program(flags)
    res = run_bass_kernel_spmd(nc, in_maps, core_ids=list(range(N_CORES)))
    out = np.stack([res.results[c]["y"] for c in range(N_CORES)], axis=0)
    return out.astype(np.float32)


# revision 29
# speedup vs baseline: 1.4355x; 1.0826x over previous
"""DPP attention-3 Trainium2 kernel.

Data-parallel across 8 NeuronCores: one batch element per core; all
weights replicated.  The reference's [B,L,L,L] det_values tensor is never
materialized: since K = s2 @ s2.T is exactly symmetric, the k-reduction of
the 3x3 determinants collapses to

    marginal[i,j] = S0*(d_i d_j - K_ij^2) - d_i a_j - a_i d_j + 2 K_ij A_ij

with  A = K diag(w) K,  a = diag(A),  d = diag(K),  S0 = sum_k w_k d_k.

Everything is computed in the transposed [j, i] orientation (score is
symmetric up to the additive mask, which is fed pre-transposed from the
host), so the exp'd scores are directly the stationary operand of the
context matmul, the softmax denominators fall out of ones-column matmuls,
and the final output projection needs no transpose at all (ctx^T is the
lhsT the natural-orientation matmul wants).

Matmul operands are kept in bf16 (fp32 PSUM accumulation): fp32 matmuls
cost 4 cycles per output row on TRN2 vs 1 for bf16, and the score scale
here (|score| < 1) makes the bf16 rounding negligible (~6e-5 final rel
err measured).  The residual/LayerNorm path stays fp32.
"""

import numpy as np

B, L, H = 8, 160, 64
N_CORES = 8
EPS = 1e-12
CHUNKS = [(0, 128), (128, 32)]  # partition chunks covering L=160

_programs = {}  # (flags..., scale) -> nc


def _build_program(use_mask, use_w, use_bde, use_ln, scale):
    import concourse.bass as bass
    import concourse.bacc as bacc_mod
    import concourse.tile as tile
    from concourse import bacc, mybir
    from concourse.masks import make_identity

    f32 = mybir.dt.float32
    bf16 = mybir.dt.bfloat16
    Alu = mybir.AluOpType
    Act = mybir.ActivationFunctionType

    nc = bacc.Bacc(
        "TRN2",
        target_bir_lowering=False,
        debug=False,
        enable_asserts=False,
        num_devices=N_CORES,
    )

    inv_h4 = float(H ** -0.25)

    xt_d = nc.dram_tensor("xt", [H, L], bf16, kind="ExternalInput").ap()
    x_d = nc.dram_tensor("x", [L, H], f32, kind="ExternalInput").ap()
    w3_d = nc.dram_tensor("w3", [H, 3 * H], bf16, kind="ExternalInput").ap()
    bqp_d = nc.dram_tensor("bqp", [H, 1], f32, kind="ExternalInput").ap()
    maskt_d = wrow_d = bde_d = lnw_d = lnb_d = None
    if use_mask:
        maskt_d = nc.dram_tensor("maskt", [L, L], f32, kind="ExternalInput").ap()
    if use_w:
        wrow_d = nc.dram_tensor("wrow", [1, L], f32, kind="ExternalInput").ap()
    if use_bde:
        bde_d = nc.dram_tensor("bde", [1, H], f32, kind="ExternalInput").ap()
    if use_ln:
        lnw_d = nc.dram_tensor("lnw", [1, H], f32, kind="ExternalInput").ap()
        lnb_d = nc.dram_tensor("lnb", [1, H], f32, kind="ExternalInput").ap()
    y_d = nc.dram_tensor("y", [L, H], f32, kind="ExternalOutput").ap()

    with tile.TileContext(nc) as tc:
        from contextlib import ExitStack

        with ExitStack() as ctx:
            con = ctx.enter_context(tc.tile_pool(name="con", bufs=1))
            wk = ctx.enter_context(tc.tile_pool(name="wk", bufs=1))
            ppb = ctx.enter_context(tc.tile_pool(name="ppb", bufs=4, space="PSUM"))
            pps = ctx.enter_context(tc.tile_pool(name="pps", bufs=2, space="PSUM"))
            ppm = ctx.enter_context(tc.tile_pool(name="ppm", bufs=2, space="PSUM"))

            # --- inputs (critical-path DMAs first) ---
            xT = con.tile([H, L], bf16)
            nc.sync.dma_start(out=xT[:], in_=xt_d)
            xc = []
            for off, p in CHUNKS:
                t = con.tile([p, H], f32, tag=f"x{off}")
                nc.sync.dma_start(out=t[:], in_=x_d[off : off + p, :])
                xc.append(t)
            w3 = con.tile([H, 3 * H], bf16)
            nc.sync.dma_start(out=w3[:], in_=w3_d)
            wqt = w3[:, 0:H]
            wvt = w3[:, H : 2 * H]
            wdt = w3[:, 2 * H : 3 * H]
            bqp = con.tile([H, 1], f32)
            nc.sync.dma_start(out=bqp[:], in_=bqp_d)

            # --- constants (gpsimd; no deps) ---
            ident_bf = con.tile([128, 128], bf16)
            make_identity(nc, ident_bf[:])
            ones64b = con.tile([H, 1], bf16)
            nc.gpsimd.memset(ones64b[:], 1.0)
            ones128b = con.tile([128, 1], bf16)
            nc.gpsimd.memset(ones128b[:], 1.0)
            onesr = con.tile([1, 128], f32)
            nc.gpsimd.memset(onesr[:], 1.0)
            epsc = con.tile([128, 1], f32)
            nc.gpsimd.memset(epsc[:], EPS)

            masktc = []
            if use_mask:
                for off, p in CHUNKS:
                    t = con.tile([p, L], f32, tag=f"mt{off}")
                    nc.sync.dma_start(out=t[:], in_=maskt_d[off : off + p, :])
                    masktc.append(t)
            if use_w:
                ident = con.tile([128, 128], f32)
                make_identity(nc, ident[:])
                wrow = con.tile([1, L], f32)
                nc.sync.dma_start(out=wrow[:], in_=wrow_d)
            if use_bde:
                bde_r = con.tile([1, H], f32)
                nc.sync.dma_start(out=bde_r[:], in_=bde_d)
            if use_ln:
                lnw_r = con.tile([1, H], f32)
                nc.sync.dma_start(out=lnw_r[:], in_=lnw_d)
                lnb_r = con.tile([1, H], f32)
                nc.sync.dma_start(out=lnb_r[:], in_=lnb_d)

            # pull the single ACT table load off the critical path
            warm = wk.tile([1, 1], f32)
            nc.gpsimd.memset(warm[:], 1.0)
            warm2 = wk.tile([1, 1], f32)
            nc.scalar.copy(warm2[:], warm[:])

            # --- sampler^2 transposed: s2T = Square(invH4*(Wq @ xT) + bq*invH4)
            qT_ps = ppb.tile([H, L], f32, tag="big")
            nc.tensor.matmul(qT_ps[:], wqt, xT[:], start=True, stop=True)
            s2T = wk.tile([H, L], bf16)
            nc.scalar.activation(s2T[:], qT_ps[:], Act.Square, bias=bqp[:], scale=inv_h4)

            # --- K chunks [p, L] ---
            Kc = []
            for i, (off, p) in enumerate(CHUNKS):
                kps = ppb.tile([p, L], f32, tag="big")
                nc.tensor.matmul(kps[:], s2T[:, off : off + p], s2T[:], start=True, stop=True)
                k_sb = wk.tile([p, L], bf16, tag=f"K{off}")
                if i == 0:
                    nc.scalar.copy(k_sb[:], kps[:])
                else:
                    nc.vector.tensor_copy(k_sb[:], kps[:])
                Kc.append(k_sb)

            # --- d = diag(K) via s4T = s2T*s2T ---
            s4T = wk.tile([H, L], bf16)
            nc.vector.tensor_mul(s4T[:], s2T[:], s2T[:])
            drow_ps = pps.tile([1, L], f32, tag="small")
            nc.tensor.matmul(drow_ps[:], ones64b[:], s4T[:], start=True, stop=True)
            drow = wk.tile([1, L], bf16)
            nc.vector.tensor_copy(drow[:], drow_ps[:])
            dcol_ps = []
            for off, p in CHUNKS:
                dps = pps.tile([p, 1], f32, tag="small")
                nc.tensor.matmul(dps[:], s4T[:, off : off + p], ones64b[:], start=True, stop=True)
                dcol_ps.append(dps)

            # S0 = sum_k w_k d_k  (scalar [1,1], fp32)
            S0_t = wk.tile([1, 1], f32)
            if use_w:
                wd_row = wk.tile([1, L], f32)
                nc.vector.tensor_mul(wd_row[:], drow[:], wrow[:])
                nc.vector.reduce_sum(S0_t[:], wd_row[:], axis=mybir.AxisListType.X)
            else:
                nc.vector.reduce_sum(S0_t[:], drow[:], axis=mybir.AxisListType.X)
            S0s_t = wk.tile([1, 1], f32)
            nc.vector.tensor_scalar(S0s_t[:], S0_t[:], scale, None, op0=Alu.mult)
            nS0s_t = wk.tile([1, 1], f32)
            nc.vector.tensor_scalar(nS0s_t[:], S0_t[:], -scale, None, op0=Alu.mult)
            # broadcast scale*S0 down partition columns per chunk
            S0scol = []
            for off, p in CHUNKS:
                sps = pps.tile([p, 1], f32, tag="small")
                nc.tensor.matmul(sps[:], onesr[0:1, 0:p], S0s_t[:], start=True, stop=True)
                scol = wk.tile([p, 1], f32, tag=f"s0c{off}")
                nc.vector.tensor_copy(scol[:], sps[:])
                S0scol.append(scol)

            wcol = [None, None]
            if use_w:
                for i, (off, p) in enumerate(CHUNKS):
                    wps = pps.tile([p, 1], f32, tag="small")
                    nc.tensor.transpose(wps[:], wrow[0:1, off : off + p], ident[0:1, 0:1])
                    wc = wk.tile([p, 1], f32, tag=f"wc{off}")
                    nc.vector.tensor_copy(wc[:], wps[:])
                    wcol[i] = wc

            # --- KK (optionally weighted) for a = diag(K diag(w) K) ---
            KKc = []
            for i, (off, p) in enumerate(CHUNKS):
                kk = wk.tile([p, L], bf16, tag=f"KK{off}")
                nc.vector.tensor_mul(kk[:], Kc[i][:], Kc[i][:])
                if use_w:
                    nc.vector.tensor_scalar(kk[:], kk[:], wcol[i][:], None, op0=Alu.mult)
                KKc.append(kk)
            arow_ps = pps.tile([1, L], f32, tag="small")
            nc.tensor.matmul(arow_ps[:], ones128b[:], KKc[0][:], start=True, stop=False)
            nc.tensor.matmul(arow_ps[:], ones128b[0:32, :], KKc[1][:], start=False, stop=True)
            arow = wk.tile([1, L], bf16)
            nc.vector.tensor_copy(arow[:], arow_ps[:])

            # --- rank-2 factors: v0 = -S0s*d + s*a ; v1 = s*d  (bf16 rows) ---
            v0_r = wk.tile([1, L], bf16)
            tmp_r = wk.tile([1, L], bf16)
            nc.vector.tensor_scalar(tmp_r[:], arow[:], scale, None, op0=Alu.mult)
            nc.vector.scalar_tensor_tensor(
                v0_r[:], drow[:], nS0s_t[:], tmp_r[:], op0=Alu.mult, op1=Alu.add
            )
            v1_r = wk.tile([1, L], bf16)
            nc.vector.tensor_scalar(v1_r[:], drow[:], scale, None, op0=Alu.mult)

            # --- A2s = -2*scale * K diag(w) K  (PSUM, per chunk) ---
            wK2s = []
            for i, (off, p) in enumerate(CHUNKS):
                t = wk.tile([p, L], bf16, tag=f"wk2{off}")
                if use_w:
                    nc.vector.tensor_scalar(
                        t[:], Kc[i][:], wcol[i][:], -2.0 * scale, op0=Alu.mult, op1=Alu.mult
                    )
                else:
                    nc.vector.tensor_scalar(t[:], Kc[i][:], -2.0 * scale, None, op0=Alu.mult)
                wK2s.append(t)

            # diag contribution as a matmul operand:
            # dsel[p, f] = -scale*d_p at f == p+off else 0  (bf16)
            dsel = []
            for i, (off, p) in enumerate(CHUNKS):
                dsc = wk.tile([p, 1], f32, tag=f"dsc{off}")
                nc.vector.tensor_scalar(dsc[:], dcol_ps[i][:], -scale, None, op0=Alu.mult)
                ds = wk.tile([p, L], bf16, tag=f"dsel{off}")
                nc.gpsimd.affine_select(
                    out=ds[:],
                    in_=dsc[:, 0:1].broadcast_to([p, L]),
                    compare_op=Alu.is_equal,
                    fill=0.0,
                    base=-off,
                    pattern=[[1, L]],
                    channel_multiplier=-1,
                )
                dsel.append(ds)

            # --- score + exp per chunk (transposed orientation) ---
            ec = []
            for i, (off, p) in enumerate(CHUNKS):
                a2s = ppb.tile([p, L], f32, tag="big")
                nc.tensor.matmul(a2s[:], Kc[0][:, off : off + p], wK2s[0][:], start=True, stop=False)
                nc.tensor.matmul(a2s[:], Kc[1][:, off : off + p], wK2s[1][:], start=False, stop=True)
                # rank-2 part plus the diagonal (identity-stationary matmul)
                rs = ppb.tile([p, L], f32, tag="big")
                nc.tensor.matmul(rs[:], drow[0:1, off : off + p], v0_r[:], start=True, stop=False)
                nc.tensor.matmul(rs[:], arow[0:1, off : off + p], v1_r[:], start=False, stop=False)
                nc.tensor.matmul(rs[:], ident_bf[0:p, 0:p], dsel[i][:], start=False, stop=True)

                t1 = wk.tile([p, L], f32, tag=f"t1{off}")
                # t1 = K*(S0*scale) + A2s
                nc.vector.scalar_tensor_tensor(
                    t1[:], Kc[i][:], S0scol[i][:], a2s[:], op0=Alu.mult, op1=Alu.add
                )
                # t1 = t1 * K   (= -scale*(2A - S0*K) ⊙ K)
                nc.vector.tensor_mul(t1[:], t1[:], Kc[i][:])
                # t1 += Rs + diag
                nc.vector.tensor_add(t1[:], t1[:], rs[:])
                if use_mask:
                    nc.vector.tensor_add(t1[:], t1[:], masktc[i][:])
                e = wk.tile([p, L], bf16, tag=f"e{off}")
                nc.scalar.activation(e[:], t1[:], Act.Exp)
                ec.append(e)

            # --- value projection (bf16) ---
            Vh = []
            for i, (off, p) in enumerate(CHUNKS):
                vps = ppm.tile([p, H], f32, tag="p64")
                nc.tensor.matmul(vps[:], xT[:, off : off + p], wvt, start=True, stop=True)
                vh = wk.tile([p, H], bf16, tag=f"vh{off}")
                nc.vector.tensor_copy(vh[:], vps[:])
                Vh.append(vh)

            # softmax denominators Z (fp32 PSUM accumulation of bf16 e)
            zcol_ps = []
            for off, p in CHUNKS:
                zps = pps.tile([p, 1], f32, tag="small")
                nc.tensor.matmul(zps[:], ec[0][:, off : off + p], ones128b[:], start=True, stop=False)
                nc.tensor.matmul(zps[:], ec[1][:, off : off + p], ones128b[0:32, :], start=False, stop=True)
                zcol_ps.append(zps)

            # ctxT [H, L] = V^T e^T
            ctxT_ps = ppb.tile([H, L], f32, tag="big")
            nc.tensor.matmul(ctxT_ps[:], Vh[0][:], ec[0][:], start=True, stop=False)
            nc.tensor.matmul(ctxT_ps[:], Vh[1][:], ec[1][:], start=False, stop=True)
            ctxT = wk.tile([H, L], bf16)
            # split the copy so chunk 0's output matmul can start early
            nc.vector.tensor_copy(ctxT[:, 0:128], ctxT_ps[:, 0:128])
            nc.vector.tensor_copy(ctxT[:, 128:160], ctxT_ps[:, 128:160])

            if use_bde:
                bde_ps = ppm.tile([128, H], f32, tag="p64")
                nc.tensor.matmul(bde_ps[:], onesr[:], bde_r[:], start=True, stop=True)
                bde_b = wk.tile([128, H], f32)
                nc.vector.tensor_copy(bde_b[:], bde_ps[:])
            if use_ln:
                lnw_ps = ppm.tile([128, H], f32, tag="p64")
                nc.tensor.matmul(lnw_ps[:], onesr[:], lnw_r[:], start=True, stop=True)
                lnw_b = wk.tile([128, H], f32)
                nc.vector.tensor_copy(lnw_b[:], lnw_ps[:])
                lnb_ps = ppm.tile([128, H], f32, tag="p64")
                nc.tensor.matmul(lnb_ps[:], onesr[:], lnb_r[:], start=True, stop=True)
                lnb_b = wk.tile([128, H], f32)
                nc.vector.tensor_copy(lnb_b[:], lnb_ps[:])

            # --- per chunk: output projection (already natural), normalize,
            # residual, LayerNorm ---
            for i, (off, p) in enumerate(CHUNKS):
                rcol = wk.tile([p, 1], f32, tag=f"rc{off}")
                nc.vector.reciprocal(rcol[:], zcol_ps[i][:])

                # out_nat[i, h'] = sum_h ctxT[h, i] * WdT[h, h']
                ops = ppm.tile([p, H], f32, tag="p64")
                nc.tensor.matmul(ops[:], ctxT[:, off : off + p], wdt, start=True, stop=True)

                res = wk.tile([p, H], f32, tag=f"res{off}")
                # res = out_nat * r + x
                nc.vector.scalar_tensor_tensor(
                    res[:], ops[:], rcol[:], xc[i][:], op0=Alu.mult, op1=Alu.add
                )
                if use_bde:
                    nc.vector.tensor_add(res[:], res[:], bde_b[0:p, :])

                stats = wk.tile([p, 6], f32, tag=f"st{off}")
                nc.vector.bn_stats(stats[:], res[:])
                mv = wk.tile([p, 2], f32, tag=f"mv{off}")
                nc.vector.bn_aggr(mv[:], stats[:])
                # rstd = exp(-0.5*ln(var+eps)); Ln+Exp live in one table set
                lnv = wk.tile([p, 1], f32, tag=f"lnv{off}")
                nc.scalar.activation(lnv[:], mv[:, 1:2], Act.Ln, bias=epsc[0:p, :])
                rstd = wk.tile([p, 1], f32, tag=f"rst{off}")
                nc.scalar.activation(rstd[:], lnv[:], Act.Exp, scale=-0.5)

                y_t = wk.tile([p, H], f32, tag=f"y{off}")
                nc.vector.tensor_scalar(
                    y_t[:], res[:], mv[:, 0:1], rstd[:], op0=Alu.subtract, op1=Alu.mult
                )
                if use_ln:
                    nc.vector.tensor_mul(y_t[:], y_t[:], lnw_b[0:p, :])
                    nc.vector.tensor_add(y_t[:], y_t[:], lnb_b[0:p, :])
                nc.sync.dma_start(out=y_d[off : off + p, :], in_=y_t[:])

    # Compile with the combined Ln+Exp activation-table set preferred, so a
    # single ACT_TABLE_LOAD covers Square/Copy/Exp/Ln (the default greedy
    # selection alternates between the exp-only and ln-only sets: 6 loads,
    # ~7.7us of ACT time).  The set *order* must be preserved — the position
    # in this dict is the act_func_set_id walrus resolves against
    # act_info.json — so instead of reordering, hide this kernel's functions
    # from every other set, forcing the selector onto the combined one at
    # its true index.
    orig_tables = bacc_mod.get_activation_tables
    mine = {Act.Exp, Act.Ln, Act.Square, Act.Copy, Act.Identity}

    def _patched(arch):
        tabs = orig_tables(arch)
        assert "natural_log_exp_and_others" in tabs
        return {
            n: (fs if n == "natural_log_exp_and_others" else fs - mine)
            for n, fs in tabs.items()
        }

    bacc_mod.get_activation_tables = _patched
    try:
        nc.compile()
    finally:
        bacc_mod.get_activation_tables = orig_tables
    return nc


def _prepare(inputs):
    import ml_dtypes

    bf = ml_dtypes.bfloat16
    x = np.ascontiguousarray(np.asarray(inputs["input_tensor"], dtype=np.float32))
    mask = np.ascontiguousarray(np.asarray(inputs["attention_mask"], dtype=np.float32))
    Wq = np.asarray(inputs["Wq"], dtype=np.float32)
    bq = np.asarray(inputs["bq"], dtype=np.float32)
    Wv = np.asarray(inputs["Wv"], dtype=np.float32)
    bv = np.asarray(inputs["bv"], dtype=np.float32)
    Wd = np.asarray(inputs["Wd"], dtype=np.float32)
    bd = np.asarray(inputs["bd"], dtype=np.float32)
    ln_w = np.asarray(inputs["ln_w"], dtype=np.float32)
    ln_b = np.asarray(inputs["ln_b"], dtype=np.float32)
    scale = np.float32(np.asarray(inputs["scale_factor"]).reshape(()))

    use_mask = bool(np.any(mask != 0.0))
    wvals = (mask[:, 0, :] > -10000.0).astype(np.float32)
    use_w = not bool(np.all(wvals == 1.0))
    bde = bd + Wd @ bv  # value bias folded through the output projection
    use_bde = bool(np.any(bde != 0.0))
    use_ln = not (bool(np.all(ln_w == 1.0)) and bool(np.all(ln_b == 0.0)))

    flags = (use_mask, use_w, use_bde, use_ln, float(scale))
    w3 = np.concatenate([Wq.T, Wv.T, Wd.T], axis=1)  # [H, 3H]
    shared = {
        "w3": np.ascontiguousarray(w3).astype(bf),
        "bqp": np.ascontiguousarray((bq * (H ** -0.25)).reshape(H, 1)),
    }
    if use_bde:
        shared["bde"] = np.ascontiguousarray(bde.reshape(1, H))
    if use_ln:
        shared["lnw"] = np.ascontiguousarray(ln_w.reshape(1, H))
        shared["lnb"] = np.ascontiguousarray(ln_b.reshape(1, H))

    in_maps = []
    for c in range(N_CORES):
        m = dict(shared)
        m["x"] = np.ascontiguousarray(x[c])
        m["xt"] = np.ascontiguousarray(x[c].T).astype(bf)
        if use_mask:
            m["maskt"] = np.ascontiguousarray(mask[c].T)
        if use_w:
            m["wrow"] = np.ascontiguousarray(wvals[c].reshape(1, L))
        in_maps.append(m)
    return flags, in_maps


def _get_program(flags):
    if flags not in _programs:
        _programs[flags] = _build_program(*flags)
    return _programs[flags]


def kernel(**inputs):
    from concourse.bass_utils import run_bass_kernel_spmd

    flags, in_maps = _prepare(inputs)
    nc = _get_program(flags)
    res = run_bass_kernel_spmd(nc, in_maps, core_ids=list(range(N_CORES)))
    out = np.stack([res.results[c]["y"] for c in range(N_CORES)], axis=0)
    return out.astype(np.float32)


# revision 36
# speedup vs baseline: 1.5314x; 1.0668x over previous
"""DPP attention-3 Trainium2 kernel.

Data-parallel across 8 NeuronCores: one batch element per core; all
weights replicated.  The reference's [B,L,L,L] det_values tensor is never
materialized: since K = s2 @ s2.T is exactly symmetric, the k-reduction of
the 3x3 determinants collapses to

    marginal[i,j] = S0*(d_i d_j - K_ij^2) - d_i a_j - a_i d_j + 2 K_ij A_ij

with  A = K diag(w) K,  a = diag(A),  d = diag(K),  S0 = sum_k w_k d_k.

Everything is computed in the transposed [j, i] orientation (score is
symmetric up to the additive mask, which is fed pre-transposed from the
host), so the exp'd scores are directly the stationary operand of the
context matmul, the softmax denominators fall out of ones-column matmuls,
and the final output projection needs no transpose at all (ctx^T is the
lhsT the natural-orientation matmul wants).

Matmul operands are kept in bf16 (fp32 PSUM accumulation): fp32 matmuls
cost 4 cycles per output row on TRN2 vs 1 for bf16, and the score scale
here (|score| < 1) makes the bf16 rounding negligible (~6e-5 final rel
err measured).  The residual/LayerNorm path stays fp32.
"""

import numpy as np

B, L, H = 8, 160, 64
N_CORES = 8
EPS = 1e-12
CHUNKS = [(0, 128), (128, 32)]  # partition chunks covering L=160

_programs = {}  # (flags..., scale) -> nc


def _build_program(use_mask, use_w, use_bde, use_ln, scale):
    import concourse.bass as bass
    import concourse.bacc as bacc_mod
    import concourse.tile as tile
    from concourse import bacc, mybir
    from concourse.masks import make_identity

    f32 = mybir.dt.float32
    bf16 = mybir.dt.bfloat16
    Alu = mybir.AluOpType
    Act = mybir.ActivationFunctionType

    nc = bacc.Bacc(
        "TRN2",
        target_bir_lowering=False,
        debug=False,
        enable_asserts=False,
        num_devices=N_CORES,
    )

    inv_h4 = float(H ** -0.25)

    xt_d = nc.dram_tensor("xt", [H, L], bf16, kind="ExternalInput").ap()
    x_d = nc.dram_tensor("x", [L, H], f32, kind="ExternalInput").ap()
    w3_d = nc.dram_tensor("w3", [H, 3 * H], bf16, kind="ExternalInput").ap()
    bqp_d = nc.dram_tensor("bqp", [H, 1], f32, kind="ExternalInput").ap()
    maskt_d = wrow_d = bde_d = lnw_d = lnb_d = None
    if use_mask:
        maskt_d = nc.dram_tensor("maskt", [L, L], f32, kind="ExternalInput").ap()
    if use_w:
        wrow_d = nc.dram_tensor("wrow", [1, L], f32, kind="ExternalInput").ap()
    if use_bde:
        bde_d = nc.dram_tensor("bde", [1, H], f32, kind="ExternalInput").ap()
    if use_ln:
        lnw_d = nc.dram_tensor("lnw", [1, H], f32, kind="ExternalInput").ap()
        lnb_d = nc.dram_tensor("lnb", [1, H], f32, kind="ExternalInput").ap()
    y_d = nc.dram_tensor("y", [L, H], f32, kind="ExternalOutput").ap()

    with tile.TileContext(nc) as tc:
        from contextlib import ExitStack

        with ExitStack() as ctx:
            con = ctx.enter_context(tc.tile_pool(name="con", bufs=1))
            wk = ctx.enter_context(tc.tile_pool(name="wk", bufs=1))
            ppb = ctx.enter_context(tc.tile_pool(name="ppb", bufs=3, space="PSUM"))
            pps = ctx.enter_context(tc.tile_pool(name="pps", bufs=3, space="PSUM"))
            ppm = ctx.enter_context(tc.tile_pool(name="ppm", bufs=2, space="PSUM"))

            # --- inputs; spread descriptor generation across engine DGE
            # queues (a single queue costs ~600ns of issue time per DMA) ---
            xT = con.tile([H, L], bf16)
            nc.scalar.dma_start(out=xT[:], in_=xt_d)
            w3 = con.tile([H, 3 * H], bf16)
            nc.sync.dma_start(out=w3[:], in_=w3_d)
            wqt = w3[:, 0:H]
            wvt = w3[:, H : 2 * H]
            wdt = w3[:, 2 * H : 3 * H]
            xc = []
            for i, (off, p) in enumerate(CHUNKS):
                t = con.tile([p, H], f32, tag=f"x{off}")
                eng = nc.sync if i == 0 else nc.gpsimd
                eng.dma_start(out=t[:], in_=x_d[off : off + p, :])
                xc.append(t)
            bqp = con.tile([H, 1], f32)
            nc.sync.dma_start(out=bqp[:], in_=bqp_d)

            # --- constants (gpsimd; no deps) ---
            ident_bf = con.tile([128, 128], bf16)
            make_identity(nc, ident_bf[:])
            ones64b = con.tile([H, 1], bf16)
            nc.gpsimd.memset(ones64b[:], 1.0)
            ones128b = con.tile([128, 1], bf16)
            nc.gpsimd.memset(ones128b[:], 1.0)
            onesr = con.tile([1, 128], f32)
            nc.gpsimd.memset(onesr[:], 1.0)
            epsc = con.tile([128, 1], f32)
            nc.gpsimd.memset(epsc[:], EPS)

            masktc = []
            if use_mask:
                for off, p in CHUNKS:
                    t = con.tile([p, L], f32, tag=f"mt{off}")
                    nc.sync.dma_start(out=t[:], in_=maskt_d[off : off + p, :])
                    masktc.append(t)
            if use_w:
                ident = con.tile([128, 128], f32)
                make_identity(nc, ident[:])
                wrow = con.tile([1, L], f32)
                nc.sync.dma_start(out=wrow[:], in_=wrow_d)
            if use_bde:
                bde_r = con.tile([1, H], f32)
                nc.sync.dma_start(out=bde_r[:], in_=bde_d)
            if use_ln:
                lnw_r = con.tile([1, H], f32)
                nc.sync.dma_start(out=lnw_r[:], in_=lnw_d)
                lnb_r = con.tile([1, H], f32)
                nc.sync.dma_start(out=lnb_r[:], in_=lnb_d)

            # pull the single ACT table load off the critical path
            warm = wk.tile([1, 1], f32)
            nc.gpsimd.memset(warm[:], 1.0)
            warm2 = wk.tile([1, 1], f32)
            nc.scalar.copy(warm2[:], warm[:])

            # --- sampler^2 transposed: s2T = Square(invH4*(Wq @ xT) + bq*invH4)
            qT_ps = ppb.tile([H, L], f32, tag="big")
            nc.tensor.matmul(qT_ps[:], wqt, xT[:], start=True, stop=True)
            s2T = wk.tile([H, L], bf16)
            nc.scalar.activation(s2T[:], qT_ps[:], Act.Square, bias=bqp[:], scale=inv_h4)

            # --- K chunks [p, L]; KK taken straight from PSUM so it does not
            # wait on the SBUF cast ---
            Kc = []
            KKc = []
            for i, (off, p) in enumerate(CHUNKS):
                kps = ppb.tile([p, L], f32, tag="big")
                nc.tensor.matmul(kps[:], s2T[:, off : off + p], s2T[:], start=True, stop=True)
                k_sb = wk.tile([p, L], bf16, tag=f"K{off}")
                if i == 0:
                    nc.scalar.copy(k_sb[:], kps[:])
                else:
                    nc.vector.tensor_copy(k_sb[:], kps[:])
                Kc.append(k_sb)
                kk = wk.tile([p, L], bf16, tag=f"KK{off}")
                nc.vector.tensor_mul(kk[:], kps[:], k_sb[:])
                KKc.append(kk)

            # --- d = diag(K) via s4T = s2T*s2T ---
            s4T = wk.tile([H, L], bf16)
            nc.vector.tensor_mul(s4T[:], s2T[:], s2T[:])
            drow_ps = pps.tile([1, L], f32, tag="small")
            nc.tensor.matmul(drow_ps[:], ones64b[:], s4T[:], start=True, stop=True)
            drow = wk.tile([1, L], bf16)
            nc.vector.tensor_copy(drow[:], drow_ps[:])
            dcol_ps = []
            for off, p in CHUNKS:
                dps = pps.tile([p, 1], f32, tag="small")
                nc.tensor.matmul(dps[:], s4T[:, off : off + p], ones64b[:], start=True, stop=True)
                dcol_ps.append(dps)

            # S0 = sum_k w_k d_k  (scalar [1,1], fp32)
            S0_t = wk.tile([1, 1], f32)
            if use_w:
                wd_row = wk.tile([1, L], f32)
                nc.vector.tensor_mul(wd_row[:], drow[:], wrow[:])
                nc.vector.reduce_sum(S0_t[:], wd_row[:], axis=mybir.AxisListType.X)
            else:
                nc.vector.reduce_sum(S0_t[:], drow[:], axis=mybir.AxisListType.X)
            S0s_t = wk.tile([1, 1], f32)
            nc.vector.tensor_scalar(S0s_t[:], S0_t[:], scale, None, op0=Alu.mult)
            nS0s_t = wk.tile([1, 1], f32)
            nc.vector.tensor_scalar(nS0s_t[:], S0_t[:], -scale, None, op0=Alu.mult)
            # scale*S0 as a diagonal matrix per chunk: the S0s*K score term is
            # folded into the A2s PSUM accumulation via an extra matmul with
            # this as the stationary operand.
            S0sI = []
            for off, p in CHUNKS:
                sps = pps.tile([p, 1], f32, tag="small")
                nc.tensor.matmul(sps[:], onesr[0:1, 0:p], S0s_t[:], start=True, stop=True)
                sdiag = wk.tile([p, p], bf16, tag=f"s0i{off}")
                nc.vector.tensor_scalar(sdiag[:], ident_bf[0:p, 0:p], sps[:], None, op0=Alu.mult)
                S0sI.append(sdiag)

            wcol = [None, None]
            if use_w:
                for i, (off, p) in enumerate(CHUNKS):
                    wps = pps.tile([p, 1], f32, tag="small")
                    nc.tensor.transpose(wps[:], wrow[0:1, off : off + p], ident[0:1, 0:1])
                    wc = wk.tile([p, 1], f32, tag=f"wc{off}")
                    nc.vector.tensor_copy(wc[:], wps[:])
                    wcol[i] = wc

            # --- a = diag(K diag(w) K) from KK row sums ---
            if use_w:
                for i in range(2):
                    nc.vector.tensor_scalar(
                        KKc[i][:], KKc[i][:], wcol[i][:], None, op0=Alu.mult
                    )
            arow_ps = pps.tile([1, L], f32, tag="small")
            nc.tensor.matmul(arow_ps[:], ones128b[:], KKc[0][:], start=True, stop=False)
            nc.tensor.matmul(arow_ps[:], ones128b[0:32, :], KKc[1][:], start=False, stop=True)
            arow = wk.tile([1, L], bf16)
            nc.vector.tensor_copy(arow[:], arow_ps[:])

            # --- rank-2 factors: v0 = -S0s*d + s*a ; v1 = s*d  (bf16 rows) ---
            v0_r = wk.tile([1, L], bf16)
            tmp_r = wk.tile([1, L], bf16)
            nc.vector.tensor_scalar(tmp_r[:], arow[:], scale, None, op0=Alu.mult)
            nc.vector.scalar_tensor_tensor(
                v0_r[:], drow[:], nS0s_t[:], tmp_r[:], op0=Alu.mult, op1=Alu.add
            )
            v1_r = wk.tile([1, L], bf16)
            nc.vector.tensor_scalar(v1_r[:], drow[:], scale, None, op0=Alu.mult)

            # diag contribution as a matmul operand:
            # dsel[p, f] = -scale*d_p at f == p+off else 0  (bf16)
            dsel = []
            for i, (off, p) in enumerate(CHUNKS):
                dsc = wk.tile([p, 1], f32, tag=f"dsc{off}")
                nc.vector.tensor_scalar(dsc[:], dcol_ps[i][:], -scale, None, op0=Alu.mult)
                ds = wk.tile([p, L], bf16, tag=f"dsel{off}")
                nc.gpsimd.affine_select(
                    out=ds[:],
                    in_=dsc[:, 0:1].broadcast_to([p, L]),
                    compare_op=Alu.is_equal,
                    fill=0.0,
                    base=-off,
                    pattern=[[1, L]],
                    channel_multiplier=-1,
                )
                dsel.append(ds)

            # rank-2 part plus the diagonal (identity-stationary matmul);
            # emitted before the A2s group so it does not gate the final adds
            rsc = []
            for i, (off, p) in enumerate(CHUNKS):
                rs = ppm.tile([p, L], f32, tag="p64")
                nc.tensor.matmul(rs[:], drow[0:1, off : off + p], v0_r[:], start=True, stop=False)
                nc.tensor.matmul(rs[:], arow[0:1, off : off + p], v1_r[:], start=False, stop=False)
                nc.tensor.matmul(rs[:], ident_bf[0:p, 0:p], dsel[i][:], start=False, stop=True)
                rsc.append(rs)

            # --- a2s' = -2*scale*K diag(w)K + S0s*K  (PSUM, per chunk) ---
            wK2s = []
            for i, (off, p) in enumerate(CHUNKS):
                t = wk.tile([p, L], bf16, tag=f"wk2{off}")
                if use_w:
                    nc.vector.tensor_scalar(
                        t[:], Kc[i][:], wcol[i][:], -2.0 * scale, op0=Alu.mult, op1=Alu.mult
                    )
                else:
                    nc.vector.tensor_scalar(t[:], Kc[i][:], -2.0 * scale, None, op0=Alu.mult)
                wK2s.append(t)

            # --- score + exp per chunk (transposed orientation) ---
            ec = []
            for i, (off, p) in enumerate(CHUNKS):
                a2s = ppb.tile([p, L], f32, tag="big")
                nc.tensor.matmul(a2s[:], Kc[0][:, off : off + p], wK2s[0][:], start=True, stop=False)
                nc.tensor.matmul(a2s[:], Kc[1][:, off : off + p], wK2s[1][:], start=False, stop=False)
                nc.tensor.matmul(a2s[:], S0sI[i][:], Kc[i][:], start=False, stop=True)

                t1 = wk.tile([p, L], f32, tag=f"t1{off}")
                # t1 = (S0s*K + A2s) ⊙ K
                nc.vector.tensor_mul(t1[:], a2s[:], Kc[i][:])
                # t1 += Rs + diag
                nc.vector.tensor_add(t1[:], t1[:], rsc[i][:])
                if use_mask:
                    nc.vector.tensor_add(t1[:], t1[:], masktc[i][:])
                e = wk.tile([p, L], bf16, tag=f"e{off}")
                nc.scalar.activation(e[:], t1[:], Act.Exp)
                ec.append(e)

            # --- value projection (bf16) ---
            Vh = []
            for i, (off, p) in enumerate(CHUNKS):
                vps = ppm.tile([p, H], f32, tag="p64")
                nc.tensor.matmul(vps[:], xT[:, off : off + p], wvt, start=True, stop=True)
                vh = wk.tile([p, H], bf16, tag=f"vh{off}")
                nc.vector.tensor_copy(vh[:], vps[:])
                Vh.append(vh)

            # softmax denominators Z (fp32 PSUM accumulation of bf16 e)
            zcol_ps = []
            for off, p in CHUNKS:
                zps = pps.tile([p, 1], f32, tag="small")
                nc.tensor.matmul(zps[:], ec[0][:, off : off + p], ones128b[:], start=True, stop=False)
                nc.tensor.matmul(zps[:], ec[1][:, off : off + p], ones128b[0:32, :], start=False, stop=True)
                zcol_ps.append(zps)

            # ctxT [H, L] = V^T e^T
            ctxT_ps = ppb.tile([H, L], f32, tag="big")
            nc.tensor.matmul(ctxT_ps[:], Vh[0][:], ec[0][:], start=True, stop=False)
            nc.tensor.matmul(ctxT_ps[:], Vh[1][:], ec[1][:], start=False, stop=True)
            ctxT = wk.tile([H, L], bf16)
            # split the copy so chunk 0's output matmul can start early
            nc.vector.tensor_copy(ctxT[:, 0:128], ctxT_ps[:, 0:128])
            nc.vector.tensor_copy(ctxT[:, 128:160], ctxT_ps[:, 128:160])

            if use_bde:
                bde_ps = ppm.tile([128, H], f32, tag="p64")
                nc.tensor.matmul(bde_ps[:], onesr[:], bde_r[:], start=True, stop=True)
                bde_b = wk.tile([128, H], f32)
                nc.vector.tensor_copy(bde_b[:], bde_ps[:])
            if use_ln:
                lnw_ps = ppm.tile([128, H], f32, tag="p64")
                nc.tensor.matmul(lnw_ps[:], onesr[:], lnw_r[:], start=True, stop=True)
                lnw_b = wk.tile([128, H], f32)
                nc.vector.tensor_copy(lnw_b[:], lnw_ps[:])
                lnb_ps = ppm.tile([128, H], f32, tag="p64")
                nc.tensor.matmul(lnb_ps[:], onesr[:], lnb_r[:], start=True, stop=True)
                lnb_b = wk.tile([128, H], f32)
                nc.vector.tensor_copy(lnb_b[:], lnb_ps[:])

            # --- per chunk: output projection (already natural), normalize,
            # residual, LayerNorm ---
            for i, (off, p) in enumerate(CHUNKS):
                rcol = wk.tile([p, 1], f32, tag=f"rc{off}")
                nc.vector.reciprocal(rcol[:], zcol_ps[i][:])

                # out_nat[i, h'] = sum_h ctxT[h, i] * WdT[h, h']
                ops = ppm.tile([p, H], f32, tag="p64")
                nc.tensor.matmul(ops[:], ctxT[:, off : off + p], wdt, start=True, stop=True)

                res = wk.tile([p, H], f32, tag=f"res{off}")
                # res = out_nat * r + x
                nc.vector.scalar_tensor_tensor(
                    res[:], ops[:], rcol[:], xc[i][:], op0=Alu.mult, op1=Alu.add
                )
                if use_bde:
                    nc.vector.tensor_add(res[:], res[:], bde_b[0:p, :])

                stats = wk.tile([p, 6], f32, tag=f"st{off}")
                nc.vector.bn_stats(stats[:], res[:])
                mv = wk.tile([p, 2], f32, tag=f"mv{off}")
                nc.vector.bn_aggr(mv[:], stats[:])
                # rstd = exp(-0.5*ln(var+eps)); Ln+Exp live in one table set
                lnv = wk.tile([p, 1], f32, tag=f"lnv{off}")
                nc.scalar.activation(lnv[:], mv[:, 1:2], Act.Ln, bias=epsc[0:p, :])
                rstd = wk.tile([p, 1], f32, tag=f"rst{off}")
                nc.scalar.activation(rstd[:], lnv[:], Act.Exp, scale=-0.5)

                y_t = wk.tile([p, H], f32, tag=f"y{off}")
                nc.vector.tensor_scalar(
                    y_t[:], res[:], mv[:, 0:1], rstd[:], op0=Alu.subtract, op1=Alu.mult
                )
                if use_ln:
                    nc.vector.tensor_mul(y_t[:], y_t[:], lnw_b[0:p, :])
                    nc.vector.tensor_add(y_t[:], y_t[:], lnb_b[0:p, :])
                nc.sync.dma_start(out=y_d[off : off + p, :], in_=y_t[:])

    # Compile with the combined Ln+Exp activation-table set preferred, so a
    # single ACT_TABLE_LOAD covers Square/Copy/Exp/Ln (the default greedy
    # selection alternates between the exp-only and ln-only sets: 6 loads,
    # ~7.7us of ACT time).  The set *order* must be preserved — the position
    # in this dict is the act_func_set_id walrus resolves against
    # act_info.json — so instead of reordering, hide this kernel's functions
    # from every other set, forcing the selector onto the combined one at
    # its true index.
    orig_tables = bacc_mod.get_activation_tables
    mine = {Act.Exp, Act.Ln, Act.Square, Act.Copy, Act.Identity}

    def _patched(arch):
        tabs = orig_tables(arch)
        assert "natural_log_exp_and_others" in tabs
        return {
            n: (fs if n == "natural_log_exp_and_others" else fs - mine)
            for n, fs in tabs.items()
        }

    bacc_mod.get_activation_tables = _patched
    try:
        nc.compile()
    finally:
        bacc_mod.get_activation_tables = orig_tables
    return nc


def _prepare(inputs):
    import ml_dtypes

    bf = ml_dtypes.bfloat16
    x = np.ascontiguousarray(np.asarray(inputs["input_tensor"], dtype=np.float32))
    mask = np.ascontiguousarray(np.asarray(inputs["attention_mask"], dtype=np.float32))
    Wq = np.asarray(inputs["Wq"], dtype=np.float32)
    bq = np.asarray(inputs["bq"], dtype=np.float32)
    Wv = np.asarray(inputs["Wv"], dtype=np.float32)
    bv = np.asarray(inputs["bv"], dtype=np.float32)
    Wd = np.asarray(inputs["Wd"], dtype=np.float32)
    bd = np.asarray(inputs["bd"], dtype=np.float32)
    ln_w = np.asarray(inputs["ln_w"], dtype=np.float32)
    ln_b = np.asarray(inputs["ln_b"], dtype=np.float32)
    scale = np.float32(np.asarray(inputs["scale_factor"]).reshape(()))

    use_mask = bool(np.any(mask != 0.0))
    wvals = (mask[:, 0, :] > -10000.0).astype(np.float32)
    use_w = not bool(np.all(wvals == 1.0))
    bde = bd + Wd @ bv  # value bias folded through the output projection
    use_bde = bool(np.any(bde != 0.0))
    use_ln = not (bool(np.all(ln_w == 1.0)) and bool(np.all(ln_b == 0.0)))

    flags = (use_mask, use_w, use_bde, use_ln, float(scale))
    w3 = np.concatenate([Wq.T, Wv.T, Wd.T], axis=1)  # [H, 3H]
    shared = {
        "w3": np.ascontiguousarray(w3).astype(bf),
        "bqp": np.ascontiguousarray((bq * (H ** -0.25)).reshape(H, 1)),
    }
    if use_bde:
        shared["bde"] = np.ascontiguousarray(bde.reshape(1, H))
    if use_ln:
        shared["lnw"] = np.ascontiguousarray(ln_w.reshape(1, H))
        shared["lnb"] = np.ascontiguousarray(ln_b.reshape(1, H))

    in_maps = []
    for c in range(N_CORES):
        m = dict(shared)
        m["x"] = np.ascontiguousarray(x[c])
        m["xt"] = np.ascontiguousarray(x[c].T).astype(bf)
        if use_mask:
            m["maskt"] = np.ascontiguousarray(mask[c].T)
        if use_w:
            m["wrow"] = np.ascontiguousarray(wvals[c].reshape(1, L))
        in_maps.append(m)
    return flags, in_maps


def _get_program(flags):
    if flags not in _programs:
        _programs[flags] = _build_program(*flags)
    return _programs[flags]


def kernel(**inputs):
    from concourse.bass_utils import run_bass_kernel_spmd

    flags, in_maps = _prepare(inputs)
    nc = _get_program(flags)
    res = run_bass_kernel_spmd(nc, in_maps, core_ids=list(range(N_CORES)))
    out = np.stack([res.results[c]["y"] for c in range(N_CORES)], axis=0)
    return out.astype(np.float32)


# revision 42
# speedup vs baseline: 1.5808x; 1.0323x over previous
"""DPP attention-3 Trainium2 kernel.

Data-parallel across 8 NeuronCores: one batch element per core; all
weights replicated.  The reference's [B,L,L,L] det_values tensor is never
materialized: since K = s2 @ s2.T is exactly symmetric, the k-reduction of
the 3x3 determinants collapses to

    marginal[i,j] = S0*(d_i d_j - K_ij^2) - d_i a_j - a_i d_j + 2 K_ij A_ij

with  A = K diag(w) K,  a = diag(A),  d = diag(K),  S0 = sum_k w_k d_k.

Everything is computed in the transposed [j, i] orientation (score is
symmetric up to the additive mask, which is fed pre-transposed from the
host), so the exp'd scores are directly the stationary operand of the
context matmul, the softmax denominators fall out of ones-column matmuls,
and the final output projection needs no transpose at all (ctx^T is the
lhsT the natural-orientation matmul wants).

Matmul operands are kept in bf16 (fp32 PSUM accumulation): fp32 matmuls
cost 4 cycles per output row on TRN2 vs 1 for bf16, and the score scale
here (|score| < 1) makes the bf16 rounding negligible (~6e-5 final rel
err measured).  The residual/LayerNorm path stays fp32.
"""

import numpy as np

B, L, H = 8, 160, 64
N_CORES = 8
EPS = 1e-12
CHUNKS = [(0, 128), (128, 32)]  # partition chunks covering L=160

_programs = {}  # (flags..., scale) -> nc


def _build_program(use_mask, use_w, use_bde, use_ln, scale):
    import concourse.bass as bass
    import concourse.bacc as bacc_mod
    import concourse.tile as tile
    from concourse import bacc, mybir
    from concourse.masks import make_identity

    f32 = mybir.dt.float32
    bf16 = mybir.dt.bfloat16
    Alu = mybir.AluOpType
    Act = mybir.ActivationFunctionType

    nc = bacc.Bacc(
        "TRN2",
        target_bir_lowering=False,
        debug=False,
        enable_asserts=False,
        num_devices=N_CORES,
    )

    inv_h4 = float(H ** -0.25)

    xt_d = nc.dram_tensor("xt", [H, L], bf16, kind="ExternalInput").ap()
    x_d = nc.dram_tensor("x", [L, H], f32, kind="ExternalInput").ap()
    w3_d = nc.dram_tensor("w3", [H, 3 * H], bf16, kind="ExternalInput").ap()
    bqp_d = nc.dram_tensor("bqp", [H, 1], f32, kind="ExternalInput").ap()
    maskt_d = wrow_d = bde_d = lnw_d = lnb_d = None
    if use_mask:
        maskt_d = nc.dram_tensor("maskt", [L, L], f32, kind="ExternalInput").ap()
    if use_w:
        wrow_d = nc.dram_tensor("wrow", [1, L], f32, kind="ExternalInput").ap()
    if use_bde:
        bde_d = nc.dram_tensor("bde", [1, H], f32, kind="ExternalInput").ap()
    if use_ln:
        lnw_d = nc.dram_tensor("lnw", [1, H], f32, kind="ExternalInput").ap()
        lnb_d = nc.dram_tensor("lnb", [1, H], f32, kind="ExternalInput").ap()
    y_d = nc.dram_tensor("y", [L, H], f32, kind="ExternalOutput").ap()

    with tile.TileContext(nc) as tc:
        from contextlib import ExitStack

        with ExitStack() as ctx:
            con = ctx.enter_context(tc.tile_pool(name="con", bufs=1))
            wk = ctx.enter_context(tc.tile_pool(name="wk", bufs=1))
            ppb = ctx.enter_context(tc.tile_pool(name="ppb", bufs=3, space="PSUM"))
            pps = ctx.enter_context(tc.tile_pool(name="pps", bufs=3, space="PSUM"))
            ppm = ctx.enter_context(tc.tile_pool(name="ppm", bufs=2, space="PSUM"))

            # --- inputs; spread descriptor generation across engine DGE
            # queues (a single queue costs ~600ns of issue time per DMA) ---
            xT = con.tile([H, L], bf16)
            nc.scalar.dma_start(out=xT[:], in_=xt_d)
            w3 = con.tile([H, 3 * H], bf16)
            nc.sync.dma_start(out=w3[:], in_=w3_d)
            wqt = w3[:, 0:H]
            wvt = w3[:, H : 2 * H]
            wdt = w3[:, 2 * H : 3 * H]
            bqp = con.tile([H, 1], f32)
            nc.sync.dma_start(out=bqp[:], in_=bqp_d)
            xc = []
            for i, (off, p) in enumerate(CHUNKS):
                t = con.tile([p, H], f32, tag=f"x{off}")
                eng = nc.sync if i == 0 else nc.gpsimd
                eng.dma_start(out=t[:], in_=x_d[off : off + p, :])
                xc.append(t)

            # --- constants (gpsimd; no deps) ---
            ident_bf = con.tile([128, 128], bf16)
            make_identity(nc, ident_bf[:])
            ones64b = con.tile([H, 1], bf16)
            nc.gpsimd.memset(ones64b[:], 1.0)
            ones128b = con.tile([128, 1], bf16)
            nc.gpsimd.memset(ones128b[:], 1.0)
            onesr = con.tile([1, 128], f32)
            nc.gpsimd.memset(onesr[:], 1.0)
            epsc = con.tile([128, 1], f32)
            nc.gpsimd.memset(epsc[:], EPS)

            masktc = []
            if use_mask:
                for off, p in CHUNKS:
                    t = con.tile([p, L], f32, tag=f"mt{off}")
                    nc.sync.dma_start(out=t[:], in_=maskt_d[off : off + p, :])
                    masktc.append(t)
            if use_w:
                ident = con.tile([128, 128], f32)
                make_identity(nc, ident[:])
                wrow = con.tile([1, L], f32)
                nc.sync.dma_start(out=wrow[:], in_=wrow_d)
            if use_bde:
                bde_r = con.tile([1, H], f32)
                nc.sync.dma_start(out=bde_r[:], in_=bde_d)
            if use_ln:
                lnw_r = con.tile([1, H], f32)
                nc.sync.dma_start(out=lnw_r[:], in_=lnw_d)
                lnb_r = con.tile([1, H], f32)
                nc.sync.dma_start(out=lnb_r[:], in_=lnb_d)

            # pull the single ACT table load off the critical path
            warm = wk.tile([1, 1], f32)
            nc.vector.memset(warm[:], 1.0)
            warm2 = wk.tile([1, 1], f32)
            nc.scalar.copy(warm2[:], warm[:])

            # --- sampler^2 transposed: s2T = Square(invH4*(Wq @ xT) + bq*invH4)
            qT_ps = ppb.tile([H, L], f32, tag="big")
            nc.tensor.matmul(qT_ps[:], wqt, xT[:], start=True, stop=True)
            s2T = wk.tile([H, L], bf16)
            nc.scalar.activation(s2T[:], qT_ps[:], Act.Square, bias=bqp[:], scale=inv_h4)

            # --- K chunks [p, L]; KK taken straight from PSUM so it does not
            # wait on the SBUF cast ---
            Kc = []
            KKc = []
            for i, (off, p) in enumerate(CHUNKS):
                kps = ppb.tile([p, L], f32, tag="big")
                nc.tensor.matmul(kps[:], s2T[:, off : off + p], s2T[:], start=True, stop=True)
                k_sb = wk.tile([p, L], bf16, tag=f"K{off}")
                if i == 0:
                    nc.scalar.copy(k_sb[:], kps[:])
                else:
                    nc.vector.tensor_copy(k_sb[:], kps[:])
                Kc.append(k_sb)
                kk = wk.tile([p, L], bf16, tag=f"KK{off}")
                nc.vector.tensor_mul(kk[:], kps[:], k_sb[:])
                KKc.append(kk)

            # --- d = diag(K) via s4T = s2T*s2T ---
            s4T = wk.tile([H, L], bf16)
            nc.vector.tensor_mul(s4T[:], s2T[:], s2T[:])
            drow_ps = pps.tile([1, L], f32, tag="small")
            nc.tensor.matmul(drow_ps[:], ones64b[:], s4T[:], start=True, stop=True)
            drow = wk.tile([1, L], bf16)
            nc.vector.tensor_copy(drow[:], drow_ps[:])
            dcol_ps = []
            for off, p in CHUNKS:
                dps = pps.tile([p, 1], f32, tag="small")
                nc.tensor.matmul(dps[:], s4T[:, off : off + p], ones64b[:], start=True, stop=True)
                dcol_ps.append(dps)

            # S0 = sum_k w_k d_k  (scalar [1,1], fp32)
            S0_t = wk.tile([1, 1], f32)
            if use_w:
                wd_row = wk.tile([1, L], f32)
                nc.vector.tensor_mul(wd_row[:], drow[:], wrow[:])
                nc.vector.reduce_sum(S0_t[:], wd_row[:], axis=mybir.AxisListType.X)
            else:
                nc.vector.reduce_sum(S0_t[:], drow[:], axis=mybir.AxisListType.X)
            S0s_t = wk.tile([1, 1], f32)
            nc.vector.tensor_scalar(S0s_t[:], S0_t[:], scale, None, op0=Alu.mult)
            nS0s_t = wk.tile([1, 1], f32)
            nc.vector.tensor_scalar(nS0s_t[:], S0_t[:], -scale, None, op0=Alu.mult)
            # scale*S0 as a diagonal matrix per chunk: the S0s*K score term is
            # folded into the A2s PSUM accumulation via an extra matmul with
            # this as the stationary operand.
            S0sI = []
            for off, p in CHUNKS:
                sps = pps.tile([p, 1], f32, tag="small")
                nc.tensor.matmul(sps[:], onesr[0:1, 0:p], S0s_t[:], start=True, stop=True)
                sdiag = wk.tile([p, p], bf16, tag=f"s0i{off}")
                nc.vector.tensor_scalar(sdiag[:], ident_bf[0:p, 0:p], sps[:], None, op0=Alu.mult)
                S0sI.append(sdiag)

            wcol = [None, None]
            if use_w:
                for i, (off, p) in enumerate(CHUNKS):
                    wps = pps.tile([p, 1], f32, tag="small")
                    nc.tensor.transpose(wps[:], wrow[0:1, off : off + p], ident[0:1, 0:1])
                    wc = wk.tile([p, 1], f32, tag=f"wc{off}")
                    nc.vector.tensor_copy(wc[:], wps[:])
                    wcol[i] = wc

            # --- a = diag(K diag(w) K) from KK row sums ---
            if use_w:
                for i in range(2):
                    nc.vector.tensor_scalar(
                        KKc[i][:], KKc[i][:], wcol[i][:], None, op0=Alu.mult
                    )
            arow_ps = pps.tile([1, L], f32, tag="small")
            nc.tensor.matmul(arow_ps[:], ones128b[:], KKc[0][:], start=True, stop=False)
            nc.tensor.matmul(arow_ps[:], ones128b[0:32, :], KKc[1][:], start=False, stop=True)
            arow = wk.tile([1, L], bf16)
            nc.vector.tensor_copy(arow[:], arow_ps[:])

            # --- rank-2 factors: v0 = -S0s*d + s*a ; v1 = s*d  (bf16 rows) ---
            v0_r = wk.tile([1, L], bf16)
            tmp_r = wk.tile([1, L], bf16)
            nc.vector.tensor_scalar(tmp_r[:], arow[:], scale, None, op0=Alu.mult)
            nc.vector.scalar_tensor_tensor(
                v0_r[:], drow[:], nS0s_t[:], tmp_r[:], op0=Alu.mult, op1=Alu.add
            )
            v1_r = wk.tile([1, L], bf16)
            nc.vector.tensor_scalar(v1_r[:], drow[:], scale, None, op0=Alu.mult)

            # diag contribution as a matmul operand:
            # dsel[p, f] = -scale*d_p at f == p+off else 0  (bf16)
            dsel = []
            for i, (off, p) in enumerate(CHUNKS):
                dsc = wk.tile([p, 1], f32, tag=f"dsc{off}")
                nc.vector.tensor_scalar(dsc[:], dcol_ps[i][:], -scale, None, op0=Alu.mult)
                ds = wk.tile([p, L], bf16, tag=f"dsel{off}")
                nc.gpsimd.affine_select(
                    out=ds[:],
                    in_=dsc[:, 0:1].broadcast_to([p, L]),
                    compare_op=Alu.is_equal,
                    fill=0.0,
                    base=-off,
                    pattern=[[1, L]],
                    channel_multiplier=-1,
                )
                dsel.append(ds)

            # rank-2 part plus the diagonal (identity-stationary matmul);
            # emitted before the A2s group so it does not gate the final adds
            rsc = []
            for i, (off, p) in enumerate(CHUNKS):
                rs = ppm.tile([p, L], f32, tag="p64")
                nc.tensor.matmul(rs[:], drow[0:1, off : off + p], v0_r[:], start=True, stop=False)
                nc.tensor.matmul(rs[:], arow[0:1, off : off + p], v1_r[:], start=False, stop=False)
                nc.tensor.matmul(rs[:], ident_bf[0:p, 0:p], dsel[i][:], start=False, stop=True)
                rsc.append(rs)

            # --- a2s' = -2*scale*K diag(w)K + S0s*K  (PSUM, per chunk) ---
            wK2s = []
            for i, (off, p) in enumerate(CHUNKS):
                t = wk.tile([p, L], bf16, tag=f"wk2{off}")
                if use_w:
                    nc.vector.tensor_scalar(
                        t[:], Kc[i][:], wcol[i][:], -2.0 * scale, op0=Alu.mult, op1=Alu.mult
                    )
                else:
                    # ACT is idle in this window; keep the DVE queue short
                    nc.scalar.mul(t[:], Kc[i][:], -2.0 * scale)
                wK2s.append(t)

            # --- score + exp per chunk (transposed orientation) ---
            ec = []
            for i, (off, p) in enumerate(CHUNKS):
                a2s = ppb.tile([p, L], f32, tag="big")
                nc.tensor.matmul(a2s[:], Kc[0][:, off : off + p], wK2s[0][:], start=True, stop=False)
                nc.tensor.matmul(a2s[:], Kc[1][:, off : off + p], wK2s[1][:], start=False, stop=False)
                nc.tensor.matmul(a2s[:], S0sI[i][:], Kc[i][:], start=False, stop=True)

                t1 = wk.tile([p, L], f32, tag=f"t1{off}")
                # t1 = (S0s*K + A2s) ⊙ K
                nc.vector.tensor_mul(t1[:], a2s[:], Kc[i][:])
                # t1 += Rs + diag
                nc.vector.tensor_add(t1[:], t1[:], rsc[i][:])
                if use_mask:
                    nc.vector.tensor_add(t1[:], t1[:], masktc[i][:])
                e = wk.tile([p, L], bf16, tag=f"e{off}")
                nc.scalar.activation(e[:], t1[:], Act.Exp)
                ec.append(e)

            # --- value projection (bf16) ---
            Vh = []
            for i, (off, p) in enumerate(CHUNKS):
                vps = ppm.tile([p, H], f32, tag="p64")
                nc.tensor.matmul(vps[:], xT[:, off : off + p], wvt, start=True, stop=True)
                vh = wk.tile([p, H], bf16, tag=f"vh{off}")
                nc.scalar.copy(vh[:], vps[:])
                Vh.append(vh)

            # softmax denominators Z (fp32 PSUM accumulation of bf16 e)
            zcol_ps = []
            for off, p in CHUNKS:
                zps = pps.tile([p, 1], f32, tag="small")
                nc.tensor.matmul(zps[:], ec[0][:, off : off + p], ones128b[:], start=True, stop=False)
                nc.tensor.matmul(zps[:], ec[1][:, off : off + p], ones128b[0:32, :], start=False, stop=True)
                zcol_ps.append(zps)

            # ctxT [H, L] = V^T e^T
            ctxT_ps = ppb.tile([H, L], f32, tag="big")
            nc.tensor.matmul(ctxT_ps[:], Vh[0][:], ec[0][:], start=True, stop=False)
            nc.tensor.matmul(ctxT_ps[:], Vh[1][:], ec[1][:], start=False, stop=True)
            ctxT = wk.tile([H, L], bf16)
            # split the copy so chunk 0's output matmul can start early
            nc.vector.tensor_copy(ctxT[:, 0:128], ctxT_ps[:, 0:128])
            nc.vector.tensor_copy(ctxT[:, 128:160], ctxT_ps[:, 128:160])

            if use_bde:
                bde_ps = ppm.tile([128, H], f32, tag="p64")
                nc.tensor.matmul(bde_ps[:], onesr[:], bde_r[:], start=True, stop=True)
                bde_b = wk.tile([128, H], f32)
                nc.vector.tensor_copy(bde_b[:], bde_ps[:])
            if use_ln:
                lnw_ps = ppm.tile([128, H], f32, tag="p64")
                nc.tensor.matmul(lnw_ps[:], onesr[:], lnw_r[:], start=True, stop=True)
                lnw_b = wk.tile([128, H], f32)
                nc.vector.tensor_copy(lnw_b[:], lnw_ps[:])
                lnb_ps = ppm.tile([128, H], f32, tag="p64")
                nc.tensor.matmul(lnb_ps[:], onesr[:], lnb_r[:], start=True, stop=True)
                lnb_b = wk.tile([128, H], f32)
                nc.vector.tensor_copy(lnb_b[:], lnb_ps[:])

            # --- per chunk: output projection (already natural), normalize,
            # residual, LayerNorm ---
            for i, (off, p) in enumerate(CHUNKS):
                rcol = wk.tile([p, 1], f32, tag=f"rc{off}")
                nc.vector.reciprocal(rcol[:], zcol_ps[i][:])

                # out_nat[i, h'] = sum_h ctxT[h, i] * WdT[h, h']
                ops = ppm.tile([p, H], f32, tag="p64")
                nc.tensor.matmul(ops[:], ctxT[:, off : off + p], wdt, start=True, stop=True)

                res = wk.tile([p, H], f32, tag=f"res{off}")
                # res = out_nat * r + x
                nc.vector.scalar_tensor_tensor(
                    res[:], ops[:], rcol[:], xc[i][:], op0=Alu.mult, op1=Alu.add
                )
                if use_bde:
                    nc.vector.tensor_add(res[:], res[:], bde_b[0:p, :])

                stats = wk.tile([p, 6], f32, tag=f"st{off}")
                nc.vector.bn_stats(stats[:], res[:])
                mv = wk.tile([p, 2], f32, tag=f"mv{off}")
                nc.vector.bn_aggr(mv[:], stats[:])
                # rstd = exp(-0.5*ln(var+eps)); Ln+Exp live in one table set
                lnv = wk.tile([p, 1], f32, tag=f"lnv{off}")
                nc.scalar.activation(lnv[:], mv[:, 1:2], Act.Ln, bias=epsc[0:p, :])
                rstd = wk.tile([p, 1], f32, tag=f"rst{off}")
                nc.scalar.activation(rstd[:], lnv[:], Act.Exp, scale=-0.5)

                y_t = wk.tile([p, H], f32, tag=f"y{off}")
                nc.vector.tensor_scalar(
                    y_t[:], res[:], mv[:, 0:1], rstd[:], op0=Alu.subtract, op1=Alu.mult
                )
                if use_ln:
                    nc.vector.tensor_mul(y_t[:], y_t[:], lnw_b[0:p, :])
                    nc.vector.tensor_add(y_t[:], y_t[:], lnb_b[0:p, :])
                nc.sync.dma_start(out=y_d[off : off + p, :], in_=y_t[:])

    # Compile with the combined Ln+Exp activation-table set preferred, so a
    # single ACT_TABLE_LOAD covers Square/Copy/Exp/Ln (the default greedy
    # selection alternates between the exp-only and ln-only sets: 6 loads,
    # ~7.7us of ACT time).  The set *order* must be preserved — the position
    # in this dict is the act_func_set_id walrus resolves against
    # act_info.json — so instead of reordering, hide this kernel's functions
    # from every other set, forcing the selector onto the combined one at
    # its true index.
    orig_tables = bacc_mod.get_activation_tables
    mine = {Act.Exp, Act.Ln, Act.Square, Act.Copy, Act.Identity}

    def _patched(arch):
        tabs = orig_tables(arch)
        assert "natural_log_exp_and_others" in tabs
        return {
            n: (fs if n == "natural_log_exp_and_others" else fs - mine)
            for n, fs in tabs.items()
        }

    bacc_mod.get_activation_tables = _patched
    try:
        nc.compile()
    finally:
        bacc_mod.get_activation_tables = orig_tables
    return nc


def _prepare(inputs):
    import ml_dtypes

    bf = ml_dtypes.bfloat16
    x = np.ascontiguousarray(np.asarray(inputs["input_tensor"], dtype=np.float32))
    mask = np.ascontiguousarray(np.asarray(inputs["attention_mask"], dtype=np.float32))
    Wq = np.asarray(inputs["Wq"], dtype=np.float32)
    bq = np.asarray(inputs["bq"], dtype=np.float32)
    Wv = np.asarray(inputs["Wv"], dtype=np.float32)
    bv = np.asarray(inputs["bv"], dtype=np.float32)
    Wd = np.asarray(inputs["Wd"], dtype=np.float32)
    bd = np.asarray(inputs["bd"], dtype=np.float32)
    ln_w = np.asarray(inputs["ln_w"], dtype=np.float32)
    ln_b = np.asarray(inputs["ln_b"], dtype=np.float32)
    scale = np.float32(np.asarray(inputs["scale_factor"]).reshape(()))

    use_mask = bool(np.any(mask != 0.0))
    wvals = (mask[:, 0, :] > -10000.0).astype(np.float32)
    use_w = not bool(np.all(wvals == 1.0))
    bde = bd + Wd @ bv  # value bias folded through the output projection
    use_bde = bool(np.any(bde != 0.0))
    use_ln = not (bool(np.all(ln_w == 1.0)) and bool(np.all(ln_b == 0.0)))

    flags = (use_mask, use_w, use_bde, use_ln, float(scale))
    w3 = np.concatenate([Wq.T, Wv.T, Wd.T], axis=1)  # [H, 3H]
    shared = {
        "w3": np.ascontiguousarray(w3).astype(bf),
        "bqp": np.ascontiguousarray((bq * (H ** -0.25)).reshape(H, 1)),
    }
    if use_bde:
        shared["bde"] = np.ascontiguousarray(bde.reshape(1, H))
    if use_ln:
        shared["lnw"] = np.ascontiguousarray(ln_w.reshape(1, H))
        shared["lnb"] = np.ascontiguousarray(ln_b.reshape(1, H))

    in_maps = []
    for c in range(N_CORES):
        m = dict(shared)
        m["x"] = np.ascontiguousarray(x[c])
        m["xt"] = np.ascontiguousarray(x[c].T).astype(bf)
        if use_mask:
            m["maskt"] = np.ascontiguousarray(mask[c].T)
        if use_w:
            m["wrow"] = np.ascontiguousarray(wvals[c].reshape(1, L))
        in_maps.append(m)
    return flags, in_maps


def _get_program(flags):
    if flags not in _programs:
        _programs[flags] = _build_program(*flags)
    return _programs[flags]


def kernel(**inputs):
    from concourse.bass_utils import run_bass_kernel_spmd

    flags, in_maps = _prepare(inputs)
    nc = _get_program(flags)
    res = run_bass_kernel_spmd(nc, in_maps, core_ids=list(range(N_CORES)))
    out = np.stack([res.results[c]["y"] for c in range(N_CORES)], axis=0)
    return out.astype(np.float32)
